# revision 7
# baseline (speedup 1.0000x reference)
"""GAT 2-layer kernel for Trainium2, 8 NeuronCores (SPMD, dst-sharded).

Strategy:
  - Destination-node sharding: core c owns nodes [c*6250, (c+1)*6250), processes all
    edges into those nodes. Edges are sorted by dst and bucketed into per-128-dst-node
    "slots"; each slot's edge list is padded to a multiple of 128 ("chunks").
  - Stage A (replicated on every core): G1[n] = [x@W1 (192) | alpha_src (3) | alpha_dst (3)]
    computed by one 128x128x198 matmul per 128-node tile, written to a per-core HBM table.
  - Layer-1 edge phase, per chunk of 128 edges: indirect-DMA gather of G1 rows by src,
    tiny indirect gather of alpha_dst by dst, one-hot matrix S (built on DVE from iota
    vs dst_rel) used as matmul lhsT to segment-reduce both the weighted features and the
    softmax denominators in one PSUM accumulation:
        psum[d, 0:192] = sum_e exp(logit_e) * xw[src_e],  psum[d, 192:195] = sum_e exp(logit_e)
    (softmax normalization is algebraically pulled out of the sum; no max-subtraction is
    needed since logits are O(5) here).
  - Per-slot epilogue: h = relu(psum[:, :192]/(denom+eps) + bias1), then PE-transpose h and
    immediately compute G2 rows [h@W2 (64) | as2 | ad2] for this core's shard.
  - One AllGather of the G2 shard (66 cols) -> G2 full table; layer 2 repeats the edge
    phase with 1 head; final relu output written per slot.
"""
import sys
import os

sys.path.insert(0, "/opt/trn_rl_repo")
import numpy as np

N = 50000
D = 128
HID = 64
H = 3
F1 = 192
F2 = 64
NCORES = 8
NPC = N // NCORES          # 6250 nodes per core
P = 128
NBLK = (NPC + P - 1) // P  # 49 slots per core
NT = (N + P - 1) // P      # 391 stage-A node tiles
NROW1 = NT * P             # 50048 G1 rows
G1W = 198                  # xw(192) | as(3) | ad(3)
G2W = 66                   # xw2(64) | as2(1) | ad2(1)
NROWC = NBLK * P           # 6272 rows per core shard
SLOPE = 0.2
EPS = 1e-16
GRP = 8                    # chunks per fused op group

_compiled = {}


def _host_prep(inputs):
    x = np.asarray(inputs["x"], dtype=np.float32)
    ei = np.asarray(inputs["edge_index"])
    W1 = np.asarray(inputs["W1"], dtype=np.float32)
    as1 = np.asarray(inputs["att_src1"], dtype=np.float32)
    ad1 = np.asarray(inputs["att_dst1"], dtype=np.float32)
    b1 = np.asarray(inputs["bias1"], dtype=np.float32)
    W2 = np.asarray(inputs["W2"], dtype=np.float32)
    as2 = np.asarray(inputs["att_src2"], dtype=np.float32)
    ad2 = np.asarray(inputs["att_dst2"], dtype=np.float32)
    b2 = np.asarray(inputs["bias2"], dtype=np.float32)

    loops = np.arange(N, dtype=np.int64)
    src = np.concatenate([ei[0].astype(np.int64), loops])
    dst = np.concatenate([ei[1].astype(np.int64), loops])
    order = np.argsort(dst, kind="stable")
    src = src[order]
    dst = dst[order]

    # per (core, slot) chunk counts; uniform across cores per slot (SPMD)
    core = dst // NPC
    rel = dst % NPC
    slot = rel // P
    counts = np.zeros((NCORES, NBLK), dtype=np.int64)
    np.add.at(counts, (core, slot), 1)
    Ks = np.ceil(counts / P).astype(np.int64).max(axis=0)  # chunks per slot
    NCH = int(Ks.sum())
    chunk_base = np.concatenate([[0], np.cumsum(Ks)])  # chunk index base per slot

    EPAD = NCH * P
    SRC = np.zeros((NCORES, EPAD), dtype=np.int32)
    DSTI = np.zeros((NCORES, EPAD), dtype=np.int32)
    DREL = np.full((NCORES, EPAD), 255.0, dtype=np.float32)
    for c in range(NCORES):
        base_node = c * NPC
        for s in range(NBLK):
            blo = base_node + s * P
            bhi = min(blo + P, base_node + NPC)
            lo = np.searchsorted(dst, blo, side="left")
            hi = np.searchsorted(dst, bhi, side="left")
            cnt = hi - lo
            pos = int(chunk_base[s]) * P
            SRC[c, pos:pos + cnt] = src[lo:hi]
            DSTI[c, pos:pos + cnt] = dst[lo:hi]
            DREL[c, pos:pos + cnt] = (dst[lo:hi] - blo).astype(np.float32)

    # device layout [128, NCH]: partition p of chunk k = edge k*128+p
    def tr(a):
        return np.ascontiguousarray(a.reshape(NCORES, NCH, P).transpose(0, 2, 1))

    SRC_t = tr(SRC)
    DREL_t = tr(DREL)
    # G2-full row of node n (after AllGather concat in core order)
    g2row_src = (SRC // NPC) * NROWC + (SRC % NPC)
    g2row_dst = (DSTI // NPC) * NROWC + (DSTI % NPC)
    L2SRC_t = tr(g2row_src.astype(np.int32))
    L2DST_t = tr(g2row_dst.astype(np.int32))
    DSTI_t = tr(DSTI)

    xT = np.zeros((D, NROW1), dtype=np.float32)
    xT[:, :N] = x.T

    A1 = np.zeros((F1, 6), dtype=np.float32)
    for h in range(H):
        A1[h * HID:(h + 1) * HID, h] = as1[h]
        A1[h * HID:(h + 1) * HID, 3 + h] = ad1[h]
    A2 = np.stack([as2[0], ad2[0]], axis=1).astype(np.float32)  # [64, 2]

    shared = {
        "xT": xT,
        "W1": np.ascontiguousarray(W1),
        "W1T": np.ascontiguousarray(W1.T),
        "A1": A1,
        "W2": np.ascontiguousarray(W2),
        "W2T": np.ascontiguousarray(W2.T),
        "A2": A2,
        "B1": np.ascontiguousarray(np.broadcast_to(b1, (P, F1))),
        "B2": np.ascontiguousarray(np.broadcast_to(b2, (P, F2))),
        "IOTA": np.ascontiguousarray(
            np.broadcast_to(np.arange(P, dtype=np.float32), (P, P))
        ),
    }
    percore = []
    for c in range(NCORES):
        percore.append({
            "SRCI": SRC_t[c],
            "DSTI": DSTI_t[c],
            "L2SRCI": L2SRC_t[c],
            "L2DSTI": L2DST_t[c],
            "DREL": DREL_t[c],
        })
    return tuple(Ks.tolist()), shared, percore


def _ap_view(ap, extra_offset, free_dims):
    """AP with same tensor/partition dim, custom free dims [[step, count], ...]."""
    import concourse.bass as bass

    return bass.AP(
        tensor=ap.tensor, offset=ap.offset + extra_offset,
        ap=[list(ap.ap[0])] + [list(d) for d in free_dims],
    )


def _build(Ks, debug_dump=False):
    import concourse.bass as bass
    import concourse.bacc as bacc
    import concourse.tile as tile
    from concourse import mybir
    from concourse.masks import make_identity
    from contextlib import ExitStack

    f32 = mybir.dt.float32
    i32 = mybir.dt.int32
    AT = mybir.ActivationFunctionType
    OP = mybir.AluOpType
    IOA = bass.IndirectOffsetOnAxis

    NCH = int(sum(Ks))
    # global chunk list: (slot, k within slot)
    chunks = [(s, k) for s in range(NBLK) for k in range(Ks[s])]

    nc = bacc.Bacc("TRN2", target_bir_lowering=False, debug=False,
                   num_devices=NCORES)

    kind_int = "ExternalOutput" if debug_dump else "Internal"

    xT = nc.dram_tensor("xT", [D, NROW1], f32, kind="ExternalInput")
    W1 = nc.dram_tensor("W1", [D, F1], f32, kind="ExternalInput")
    W1T = nc.dram_tensor("W1T", [F1, D], f32, kind="ExternalInput")
    A1 = nc.dram_tensor("A1", [F1, 6], f32, kind="ExternalInput")
    W2 = nc.dram_tensor("W2", [F1, F2], f32, kind="ExternalInput")
    W2T = nc.dram_tensor("W2T", [F2, F1], f32, kind="ExternalInput")
    A2 = nc.dram_tensor("A2", [F2, 2], f32, kind="ExternalInput")
    B1 = nc.dram_tensor("B1", [P, F1], f32, kind="ExternalInput")
    B2 = nc.dram_tensor("B2", [P, F2], f32, kind="ExternalInput")
    IOTA = nc.dram_tensor("IOTA", [P, P], f32, kind="ExternalInput")
    SRCI = nc.dram_tensor("SRCI", [P, NCH], i32, kind="ExternalInput")
    DSTI = nc.dram_tensor("DSTI", [P, NCH], i32, kind="ExternalInput")
    L2SRCI = nc.dram_tensor("L2SRCI", [P, NCH], i32, kind="ExternalInput")
    L2DSTI = nc.dram_tensor("L2DSTI", [P, NCH], i32, kind="ExternalInput")
    DREL = nc.dram_tensor("DREL", [P, NCH], f32, kind="ExternalInput")
    OUT = nc.dram_tensor("out", [NROWC, F2], f32, kind="ExternalOutput")

    G1 = nc.dram_tensor("G1", [NROW1, G1W], f32, kind=kind_int)
    G2L = nc.dram_tensor("G2L", [NROWC, G2W], f32, kind=kind_int)
    # collective output must stay Internal (collectives reject I/O tensors)
    G2F = nc.dram_tensor("G2F", [NROWC * NCORES, G2W], f32, addr_space="Shared",
                         kind="Internal")

    with tile.TileContext(nc) as tc, ExitStack() as ctx:
        consts = ctx.enter_context(tc.tile_pool(name="consts", bufs=1))
        sbA = ctx.enter_context(tc.tile_pool(name="sbA", bufs=4))
        psum = ctx.enter_context(tc.tile_pool(name="psum", bufs=3, space="PSUM"))
        pst = ctx.enter_context(tc.tile_pool(name="pst", bufs=2, space="PSUM"))
        gpool = ctx.enter_context(tc.tile_pool(name="gpool", bufs=3))
        fpool = ctx.enter_context(tc.tile_pool(name="fpool", bufs=3))
        spool = ctx.enter_context(tc.tile_pool(name="spool", bufs=3))
        ipool = ctx.enter_context(tc.tile_pool(name="ipool", bufs=3))
        epool = ctx.enter_context(tc.tile_pool(name="epool", bufs=3))

        # ---------------- constants / weight prep ----------------
        iota = consts.tile([P, P], f32)
        nc.sync.dma_start(out=iota[:], in_=IOTA[:])
        b1t = consts.tile([P, F1], f32)
        nc.sync.dma_start(out=b1t[:], in_=B1[:])
        b2t = consts.tile([P, F2], f32)
        nc.sync.dma_start(out=b2t[:], in_=B2[:])
        ident = consts.tile([P, P], f32)
        make_identity(nc, ident[:])

        rhs1 = consts.tile([P, G1W], f32)
        nc.sync.dma_start(out=rhs1[:, :F1], in_=W1[:])
        w1t_a = consts.tile([P, D], f32)
        nc.sync.dma_start(out=w1t_a[:], in_=W1T[0:P, :])
        w1t_b = consts.tile([F1 - P, D], f32)
        nc.sync.dma_start(out=w1t_b[:], in_=W1T[P:F1, :])
        a1_a = consts.tile([P, 6], f32)
        nc.sync.dma_start(out=a1_a[:], in_=A1[0:P, :])
        a1_b = consts.tile([F1 - P, 6], f32)
        nc.sync.dma_start(out=a1_b[:], in_=A1[P:F1, :])
        pu = pst.tile([P, 8], f32, tag="tr")
        nc.tensor.matmul(out=pu[:, :6], lhsT=w1t_a[:], rhs=a1_a[:],
                         start=True, stop=False)
        nc.tensor.matmul(out=pu[:, :6], lhsT=w1t_b[:], rhs=a1_b[:],
                         start=False, stop=True)
        nc.vector.tensor_copy(out=rhs1[:, F1:F1 + 6], in_=pu[:, :6])

        w2t = consts.tile([F2, F1], f32)
        nc.sync.dma_start(out=w2t[:], in_=W2T[:])
        a2t = consts.tile([F2, 2], f32)
        nc.sync.dma_start(out=a2t[:], in_=A2[:])
        rhs2_lo = consts.tile([P, G2W], f32)
        nc.sync.dma_start(out=rhs2_lo[:, :F2], in_=W2[0:P, :])
        rhs2_hi = consts.tile([F1 - P, G2W], f32)
        nc.sync.dma_start(out=rhs2_hi[:, :F2], in_=W2[P:F1, :])
        pu2a = pst.tile([P, 8], f32, tag="tr")
        nc.tensor.matmul(out=pu2a[:, :2], lhsT=w2t[:, 0:P], rhs=a2t[:],
                         start=True, stop=True)
        nc.vector.tensor_copy(out=rhs2_lo[:, F2:F2 + 2], in_=pu2a[:, :2])
        pu2b = pst.tile([F1 - P, 8], f32, tag="tr2")
        nc.tensor.matmul(out=pu2b[:, :2], lhsT=w2t[:, P:F1], rhs=a2t[:],
                         start=True, stop=True)
        nc.vector.tensor_copy(out=rhs2_hi[:, F2:F2 + 2], in_=pu2b[:, :2])

        # ---------------- stage A: G1 = [x@W1 | as | ad] ----------------
        for t in range(NT):
            xt = sbA.tile([P, P], f32, tag="xt")
            nc.sync.dma_start(out=xt[:], in_=xT[:, t * P:(t + 1) * P])
            pa = psum.tile([P, 200], f32, tag="mm")
            nc.tensor.matmul(out=pa[:, :G1W], lhsT=xt[:], rhs=rhs1[:],
                             start=True, stop=True)
            ga = sbA.tile([P, G1W], f32, tag="ga")
            nc.vector.tensor_copy(out=ga[:], in_=pa[:, :G1W])
            nc.sync.dma_start(out=G1[t * P:(t + 1) * P, :], in_=ga[:])

        # ---------------- layer 1 edge phase ----------------
        def edge_layer(GT, width, nfeat, srci_d, dsti_d, ps_width,
                       slot_epilogue):
            """Generic per-layer edge processing.
            GT: gather table dram tensor [rows, width]; nfeat: feature cols;
            ps_width = nfeat + nheads; alpha_dst sits at GT cols [nfeat+nh:]."""
            nheads = ps_width - nfeat
            ps_cur = [None]

            g0 = 0
            while g0 < NCH:
                gsz = min(GRP, NCH - g0)
                si = ipool.tile([P, GRP], i32, tag="si")
                nc.sync.dma_start(out=si[:, :gsz], in_=srci_d[:, g0:g0 + gsz])
                di = ipool.tile([P, GRP], i32, tag="di")
                nc.sync.dma_start(out=di[:, :gsz], in_=dsti_d[:, g0:g0 + gsz])
                dr = ipool.tile([P, GRP], f32, tag="dr")
                nc.sync.dma_start(out=dr[:, :gsz], in_=DREL[:, g0:g0 + gsz])

                grow = gpool.tile([P, GRP * width], f32, tag="grow")
                adt = ipool.tile([P, GRP * nheads], f32, tag="adt")
                for j in range(gsz):
                    nc.gpsimd.indirect_dma_start(
                        out=grow[:, j * width:(j + 1) * width], out_offset=None,
                        in_=GT[:],
                        in_offset=IOA(ap=si[:, j:j + 1], axis=0))
                    nc.gpsimd.indirect_dma_start(
                        out=adt[:, j * nheads:(j + 1) * nheads], out_offset=None,
                        in_=GT[:],
                        in_offset=IOA(ap=di[:, j:j + 1], axis=0),
                        element_offset=nfeat + nheads)
                # one-hot S for the whole group: S[p, j, d] = (drel[p,j] == d)
                S8 = spool.tile([P, GRP * P], f32, tag="s8")
                nc.vector.tensor_tensor(
                    out=_ap_view(S8[:], 0, [[P, gsz], [1, P]]),
                    in0=_ap_view(dr[:], 0, [[1, gsz], [0, P]]),
                    in1=_ap_view(iota[:], 0, [[0, gsz], [1, P]]),
                    op=OP.is_equal)
                # logits: t = as + ad ; lrelu ; exp -> F[:, nfeat:nfeat+nh]
                t8 = epool.tile([P, GRP * nheads], f32, tag="t8")
                nc.vector.tensor_tensor(
                    out=_ap_view(t8[:], 0, [[nheads, gsz], [1, nheads]]),
                    in0=_ap_view(grow[:], nfeat, [[width, gsz], [1, nheads]]),
                    in1=_ap_view(adt[:], 0, [[nheads, gsz], [1, nheads]]),
                    op=OP.add)
                r8 = epool.tile([P, GRP * nheads], f32, tag="r8")
                nc.vector.tensor_scalar(
                    out=r8[:, :gsz * nheads], in0=t8[:, :gsz * nheads],
                    scalar1=0.0, scalar2=SLOPE, op0=OP.min, op1=OP.mult)
                l8 = epool.tile([P, GRP * nheads], f32, tag="l8")
                nc.vector.scalar_tensor_tensor(
                    out=l8[:, :gsz * nheads], in0=t8[:, :gsz * nheads],
                    scalar=0.0, in1=r8[:, :gsz * nheads],
                    op0=OP.max, op1=OP.add)
                F8 = fpool.tile([P, GRP * width], f32, tag="f8")
                nc.scalar.activation(
                    out=_ap_view(F8[:], nfeat, [[width, gsz], [1, nheads]]),
                    in_=_ap_view(l8[:], 0, [[nheads, gsz], [1, nheads]]),
                    func=AT.Exp)
                # F[:, :nfeat] = grow[:, :nfeat] * ex (per-head broadcast)
                hd = nfeat // nheads
                nc.vector.tensor_tensor(
                    out=_ap_view(F8[:], 0, [[width, gsz], [hd, nheads], [1, hd]]),
                    in0=_ap_view(grow[:], 0, [[width, gsz], [hd, nheads], [1, hd]]),
                    in1=_ap_view(F8[:], nfeat, [[width, gsz], [1, nheads], [0, hd]]),
                    op=OP.mult)
                # per-chunk scatter matmuls, accumulated per slot in PSUM
                for j in range(gsz):
                    s, k = chunks[g0 + j]
                    if k == 0:
                        ps_cur[0] = psum.tile([P, 200], f32, tag="mm",
                                              name="ps_slot")
                    nc.tensor.matmul(
                        out=ps_cur[0][:, :ps_width],
                        lhsT=S8[:, j * P:(j + 1) * P],
                        rhs=F8[:, j * width:j * width + ps_width],
                        start=(k == 0), stop=(k == Ks[s] - 1))
                    if k == Ks[s] - 1:
                        slot_epilogue(s, ps_cur[0])
                g0 += gsz

        # L1 slot epilogue: normalize, bias, relu -> h; transpose; G2 rows
        def epi1(s, ps):
            rc = epool.tile([P, H], f32, tag="rc")
            nc.vector.tensor_scalar_add(out=rc[:], in0=ps[:, F1:F1 + H],
                                        scalar1=EPS)
            rc2 = epool.tile([P, H], f32, tag="rc2")
            nc.vector.reciprocal(out=rc2[:], in_=rc[:])
            hm = epool.tile([P, F1], f32, tag="hm")
            nc.vector.tensor_tensor(
                out=_ap_view(hm[:], 0, [[HID, H], [1, HID]]),
                in0=_ap_view(ps[:, :F1], 0, [[HID, H], [1, HID]]),
                in1=_ap_view(rc2[:], 0, [[1, H], [0, HID]]),
                op=OP.mult)
            hb = epool.tile([P, F1], f32, tag="hb")
            nc.vector.tensor_tensor(out=hb[:], in0=hm[:], in1=b1t[:], op=OP.add)
            hr = epool.tile([P, F1], f32, tag="hr")
            nc.scalar.activation(out=hr[:], in_=hb[:], func=AT.Relu)
            # stage B: transpose h block, G2 rows
            pt1 = pst.tile([P, P], f32, tag="tr")
            nc.tensor.transpose(out=pt1[:], in_=hr[:, :P], identity=ident[:])
            pt2 = pst.tile([F1 - P, P], f32, tag="tr2")
            nc.tensor.transpose(out=pt2[:], in_=hr[:, P:F1], identity=ident[:])
            ht1 = epool.tile([P, P], f32, tag="ht1")
            nc.vector.tensor_copy(out=ht1[:], in_=pt1[:])
            ht2 = epool.tile([F1 - P, P], f32, tag="ht2")
            nc.vector.tensor_copy(out=ht2[:], in_=pt2[:])
            pg = psum.tile([P, 200], f32, tag="mm")
            nc.tensor.matmul(out=pg[:, :G2W], lhsT=ht1[:], rhs=rhs2_lo[:],
                             start=True, stop=False)
            nc.tensor.matmul(out=pg[:, :G2W], lhsT=ht2[:], rhs=rhs2_hi[:],
                             start=False, stop=True)
            g2 = epool.tile([P, G2W], f32, tag="g2")
            nc.vector.tensor_copy(out=g2[:], in_=pg[:, :G2W])
            nc.sync.dma_start(out=G2L[s * P:(s + 1) * P, :], in_=g2[:])

        edge_layer(G1, G1W, F1, SRCI, DSTI, F1 + H, epi1)

        # ---------------- AllGather G2 ----------------
        nc.gpsimd.collective_compute(
            "AllGather", mybir.AluOpType.bypass,
            replica_groups=[list(range(NCORES))],
            ins=[G2L.ap().opt()], outs=[G2F.ap().opt()])

        # ---------------- layer 2 edge phase ----------------
        def epi2(s, ps):
            rc = epool.tile([P, 1], f32, tag="rcB")
            nc.vector.tensor_scalar_add(out=rc[:], in0=ps[:, F2:F2 + 1],
                                        scalar1=EPS)
            rc2 = epool.tile([P, 1], f32, tag="rcB2")
            nc.vector.reciprocal(out=rc2[:], in_=rc[:])
            om = epool.tile([P, F2], f32, tag="om")
            nc.vector.tensor_tensor(out=om[:], in0=ps[:, :F2],
                                    in1=rc2[:].to_broadcast([P, F2]),
                                    op=OP.mult)
            ob = epool.tile([P, F2], f32, tag="ob")
            nc.vector.tensor_tensor(out=ob[:], in0=om[:], in1=b2t[:], op=OP.add)
            orl = epool.tile([P, F2], f32, tag="orl")
            nc.scalar.activation(out=orl[:], in_=ob[:], func=AT.Relu)
            nc.sync.dma_start(out=OUT[s * P:(s + 1) * P, :], in_=orl[:])

        edge_layer(G2F, G2W, F2, L2SRCI, L2DSTI, F2 + 1, epi2)

    nc.compile()
    return nc


def _get_compiled(Ks, debug_dump=False):
    key = (Ks, debug_dump)
    if key not in _compiled:
        _compiled[key] = _build(list(Ks), debug_dump=debug_dump)
    return _compiled[key]


def run(inputs, debug_dump=False, **runkw):
    from concourse import bass_utils

    Ks, shared, percore = _host_prep(inputs)
    nc = _get_compiled(Ks, debug_dump=debug_dump)
    in_maps = []
    for c in range(NCORES):
        m = dict(shared)
        m.update(percore[c])
        in_maps.append(m)
    res = bass_utils.run_bass_kernel_spmd(
        nc, in_maps, core_ids=list(range(NCORES)), **runkw)
    return res


def assemble(results):
    out = np.empty((N, F2), dtype=np.float32)
    for c in range(NCORES):
        out[c * NPC:(c + 1) * NPC] = results[c]["out"][:NPC]
    return out


def kernel(**inputs):
    res = run(inputs)
    return assemble(res.results)


# revision 20
# speedup vs baseline: 1.4653x; 1.4653x over previous
"""GAT 2-layer kernel for Trainium2, 8 NeuronCores (SPMD, dst-sharded).

Strategy (v3):
  - Destination-node sharding: core c owns nodes [c*6250,(c+1)*6250); edges bucketed
    into per-128-dst-node "slots", padded to 128-edge chunks.
  - Stage A (replicated): per 128-node tile one matmul computes
    [x@W1 (192) | alpha_src (3) | alpha_dst (3)]; xw+as go to a bf16 gather table
    G1 (512B rows: 192 bf16 xw + 3 f32 alpha_src bit-packed + pad), ad to slim f32
    table AD1. G1 is split into two <=32768-row tensors (dma_gather int16 index
    limit, 16MiB ucode offset limit).
  - Edge phase per layer: per <=8-chunk group one dma_gather (1024 row gathers/op)
    pulls source rows; one-hot S (DVE is_equal vs iota) segment-reduces
    exp(logit)-weighted features AND the softmax denominators in one per-slot PSUM
    accumulation (normalization pulled out of the sum; logits are O(5), no
    max-subtraction needed). alpha_dst[dst] is expanded edge-wise on the PE:
    one-hot-transpose S_T (built by K=1 ones-matmul broadcast of dst_rel + DVE
    compare) times the slot's alpha_dst block (gathered once per slot).
  - Per-slot epilogue: h = relu(sum/(denom+eps) + bias1); PE-transpose h and
    immediately emit G2 rows [h@W2 (64) bf16 | as2 f32] and slim AD2; AllGather
    both; layer 2 repeats the edge phase (1 head) against G2F views.
"""
import sys

sys.path.insert(0, "/opt/trn_rl_repo")
import numpy as np
import ml_dtypes

N = 50000
D = 128
HID = 64
H = 3
F1 = 192
F2 = 64
NCORES = 8
NPC = N // NCORES          # 6250 nodes per core
P = 128
NBLK = (NPC + P - 1) // P  # 49 slots per core
NT = (N + P - 1) // P      # 391 stage-A node tiles
NROW1 = NT * P             # 50048 G1 rows
HALF = 32768               # dma_gather int16 index limit
G1W = 256                  # bf16 cols: xw(192) | as f32 x3 (bf16 192:198) | pad
G2W = 128                  # bf16 cols: xw2(64) | as2 f32 (bf16 64:66) | pad
NROWC = NBLK * P           # 6272 rows per core shard
SLOPE = 0.2
EPS = 1e-16
GRP = 8                    # max chunks per dma_gather / op group
SUB = 4                    # chunks per S_T broadcast matmul (512 psum cols)

_compiled = {}


def _chunkize(src_key, dst, order_all):
    """Bucket edges per (core, slot), split by src_key half, pad to 128.
    Returns per-core flat arrays + compile-time chunk structure (shared)."""
    core = dst // NPC
    rel = dst % NPC
    slot = rel // P
    half = (src_key >= HALF).astype(np.int64)
    # counts[core, slot, half]
    counts = np.zeros((NCORES, NBLK, 2), dtype=np.int64)
    np.add.at(counts, (core, slot, half), 1)
    Ka = np.ceil(counts[:, :, 0] / P).astype(np.int64).max(axis=0)
    Kb = np.ceil(counts[:, :, 1] / P).astype(np.int64).max(axis=0)
    return Ka, Kb


def _host_prep(inputs):
    x = np.asarray(inputs["x"], dtype=np.float32)
    ei = np.asarray(inputs["edge_index"])
    W1 = np.asarray(inputs["W1"], dtype=np.float32)
    as1 = np.asarray(inputs["att_src1"], dtype=np.float32)
    ad1 = np.asarray(inputs["att_dst1"], dtype=np.float32)
    b1 = np.asarray(inputs["bias1"], dtype=np.float32)
    W2 = np.asarray(inputs["W2"], dtype=np.float32)
    as2 = np.asarray(inputs["att_src2"], dtype=np.float32)
    ad2 = np.asarray(inputs["att_dst2"], dtype=np.float32)
    b2 = np.asarray(inputs["bias2"], dtype=np.float32)

    loops = np.arange(N, dtype=np.int64)
    src = np.concatenate([ei[0].astype(np.int64), loops])
    dst = np.concatenate([ei[1].astype(np.int64), loops])
    order = np.argsort(dst, kind="stable")
    src = src[order]
    dst = dst[order]
    g2row = (src // NPC) * NROWC + (src % NPC)

    # chunk structure per layer (uniform across cores)
    Ka1, Kb1 = _chunkize(src, dst, None)
    Ka2, Kb2 = _chunkize(g2row, dst, None)

    def build_layer(key):
        Ka, Kb = (Ka1, Kb1) if key == 1 else (Ka2, Kb2)
        skey = src if key == 1 else g2row
        NCH = int((Ka + Kb).sum())
        # chunk meta: (slot, k_in_slot, table) in processing order
        meta = []
        for s in range(NBLK):
            k = 0
            for _ in range(int(Ka[s])):
                meta.append((s, k, 0)); k += 1
            for _ in range(int(Kb[s])):
                meta.append((s, k, 1)); k += 1
        # gather ops: runs of <=GRP same-table consecutive chunks
        ops = []   # (chunk_start, n_chunks, table)
        i = 0
        while i < NCH:
            t = meta[i][2]
            j = i
            while j < NCH and j - i < GRP and meta[j][2] == t:
                j += 1
            ops.append((i, j - i, t))
            i = j
        NOPS = len(ops)

        EPAD = NCH * P
        SRCK = np.zeros((NCORES, EPAD), dtype=np.int64)
        DREL = np.full((NCORES, EPAD), 255.0, dtype=np.float32)
        for c in range(NCORES):
            base_node = c * NPC
            # per-slot edge ranges (dst-sorted => contiguous)
            cb = 0
            for s in range(NBLK):
                blo = base_node + s * P
                bhi = min(blo + P, base_node + NPC)
                lo = np.searchsorted(dst, blo, side="left")
                hi = np.searchsorted(dst, bhi, side="left")
                sk = skey[lo:hi]
                dr = (dst[lo:hi] - blo).astype(np.float32)
                a_mask = sk < HALF
                for which, KK, pad in ((a_mask, Ka[s], 0),
                                       (~a_mask, Kb[s], HALF)):
                    cnt = int(which.sum())
                    pos = cb * P
                    SRCK[c, pos:pos + cnt] = sk[which]
                    # pad indices must stay valid for the table half
                    SRCK[c, pos + cnt:(cb + int(KK)) * P] = pad
                    DREL[c, pos:pos + cnt] = dr[which]
                    cb += int(KK)
        # device arrays
        DREL_t = np.ascontiguousarray(
            DREL.reshape(NCORES, NCH, P).transpose(0, 2, 1))
        DRELT = np.ascontiguousarray(DREL.reshape(NCORES, 1, EPAD))
        # wrapped int16 indices per gather op, [128, NOPS*64]
        IDXW = np.zeros((NCORES, P, NOPS * GRP * 8), dtype=np.int16)
        for c in range(NCORES):
            for o, (c0, ncg, t) in enumerate(ops):
                iv = SRCK[c, c0 * P:(c0 + ncg) * P] - (HALF if t else 0)
                w = iv.reshape(-1, 16).T.astype(np.int16)  # [16, n/16]
                IDXW[c, :, o * GRP * 8: o * GRP * 8 + w.shape[1]] = \
                    np.tile(w, (8, 1))
        return dict(NCH=NCH, meta=meta, ops=ops, NOPS=NOPS,
                    Ktot=[int(Ka[s] + Kb[s]) for s in range(NBLK)],
                    DREL=DREL_t, DRELT=DRELT, IDXW=IDXW)

    L1 = build_layer(1)
    L2 = build_layer(2)

    # per-slot block-node gather indices (alpha_dst blocks)
    BLKI = np.zeros((NCORES, P, NBLK), dtype=np.int32)
    BLKI2 = np.zeros((NCORES, P, NBLK), dtype=np.int32)
    for c in range(NCORES):
        for s in range(NBLK):
            nodes = np.minimum(c * NPC + s * P + np.arange(P), N - 1)
            BLKI[c, :, s] = nodes
            BLKI2[c, :, s] = (nodes // NPC) * NROWC + (nodes % NPC)

    xT = np.zeros((D, NROW1), dtype=np.float32)
    xT[:, :N] = x.T
    A1 = np.zeros((F1, 6), dtype=np.float32)
    for h in range(H):
        A1[h * HID:(h + 1) * HID, h] = as1[h]
        A1[h * HID:(h + 1) * HID, 3 + h] = ad1[h]
    A2 = np.stack([as2[0], ad2[0]], axis=1).astype(np.float32)

    shared = {
        "xT": xT,
        "W1": np.ascontiguousarray(W1),
        "W1T": np.ascontiguousarray(W1.T),
        "A1": A1,
        "W2": np.ascontiguousarray(W2),
        "W2T": np.ascontiguousarray(W2.T),
        "A2": A2,
        "B1": np.ascontiguousarray(np.broadcast_to(b1, (P, F1))),
        "B2": np.ascontiguousarray(np.broadcast_to(b2, (P, F2))),
        "IOTA": np.ascontiguousarray(
            np.broadcast_to(np.arange(P, dtype=np.float32), (P, P))),
        "IOTAC": np.arange(P, dtype=np.float32).reshape(P, 1),
    }
    percore = []
    for c in range(NCORES):
        percore.append({
            "DREL1": L1["DREL"][c], "DRELT1": L1["DRELT"][c],
            "IDXW1": L1["IDXW"][c],
            "DREL2": L2["DREL"][c], "DRELT2": L2["DRELT"][c],
            "IDXW2": L2["IDXW"][c],
            "BLKI": BLKI[c], "BLKI2": BLKI2[c],
        })
    key = (tuple(L1["Ktot"]), tuple(x[0] for x in L1["ops"]),
           tuple(x[1] for x in L1["ops"]), tuple(x[2] for x in L1["ops"]),
           tuple(L2["Ktot"]), tuple(x[0] for x in L2["ops"]),
           tuple(x[1] for x in L2["ops"]), tuple(x[2] for x in L2["ops"]))
    return key, (L1, L2), shared, percore


def _ap_view(ap, extra_offset, free_dims):
    import concourse.bass as bass

    return bass.AP(
        tensor=ap.tensor, offset=ap.offset + extra_offset,
        ap=[list(ap.ap[0])] + [list(d) for d in free_dims],
    )


def _build(L1, L2):
    import concourse.bass as bass
    import concourse.bacc as bacc
    import concourse.tile as tile
    from concourse import mybir
    from concourse.masks import make_identity
    from concourse.library_config import mlp
    from contextlib import ExitStack

    f32 = mybir.dt.float32
    bf16 = mybir.dt.bfloat16
    i32 = mybir.dt.int32
    i16 = mybir.dt.int16
    AT = mybir.ActivationFunctionType
    OP = mybir.AluOpType
    IOA = bass.IndirectOffsetOnAxis

    nc = bacc.Bacc("TRN2", target_bir_lowering=False, debug=False,
                   num_devices=NCORES)

    xT = nc.dram_tensor("xT", [D, NROW1], f32, kind="ExternalInput")
    W1 = nc.dram_tensor("W1", [D, F1], f32, kind="ExternalInput")
    W1T = nc.dram_tensor("W1T", [F1, D], f32, kind="ExternalInput")
    A1 = nc.dram_tensor("A1", [F1, 6], f32, kind="ExternalInput")
    W2 = nc.dram_tensor("W2", [F1, F2], f32, kind="ExternalInput")
    W2T = nc.dram_tensor("W2T", [F2, F1], f32, kind="ExternalInput")
    A2 = nc.dram_tensor("A2", [F2, 2], f32, kind="ExternalInput")
    B1 = nc.dram_tensor("B1", [P, F1], f32, kind="ExternalInput")
    B2 = nc.dram_tensor("B2", [P, F2], f32, kind="ExternalInput")
    IOTA = nc.dram_tensor("IOTA", [P, P], f32, kind="ExternalInput")
    IOTAC = nc.dram_tensor("IOTAC", [P, 1], f32, kind="ExternalInput")
    DREL1 = nc.dram_tensor("DREL1", [P, L1["NCH"]], f32, kind="ExternalInput")
    DRELT1 = nc.dram_tensor("DRELT1", [1, L1["NCH"] * P], f32,
                            kind="ExternalInput")
    IDXW1 = nc.dram_tensor("IDXW1", [P, L1["NOPS"] * GRP * 8], i16,
                           kind="ExternalInput")
    DREL2 = nc.dram_tensor("DREL2", [P, L2["NCH"]], f32, kind="ExternalInput")
    DRELT2 = nc.dram_tensor("DRELT2", [1, L2["NCH"] * P], f32,
                            kind="ExternalInput")
    IDXW2 = nc.dram_tensor("IDXW2", [P, L2["NOPS"] * GRP * 8], i16,
                           kind="ExternalInput")
    BLKI = nc.dram_tensor("BLKI", [P, NBLK], i32, kind="ExternalInput")
    BLKI2 = nc.dram_tensor("BLKI2", [P, NBLK], i32, kind="ExternalInput")
    OUT = nc.dram_tensor("out", [NROWC, F2], f32, kind="ExternalOutput")

    G1a = nc.dram_tensor("G1a", [HALF, G1W], bf16, kind="Internal")
    G1b = nc.dram_tensor("G1b", [NROW1 - HALF, G1W], bf16, kind="Internal")
    AD1 = nc.dram_tensor("AD1", [NROW1, 4], f32, kind="Internal")
    G2L = nc.dram_tensor("G2L", [NROWC, G2W], bf16, kind="Internal")
    AD2L = nc.dram_tensor("AD2L", [NROWC, 2], f32, kind="Internal")
    G2F = nc.dram_tensor("G2F", [NROWC * NCORES, G2W], bf16,
                         addr_space="Shared", kind="Internal")
    AD2F = nc.dram_tensor("AD2F", [NROWC * NCORES, 2], f32,
                          addr_space="Shared", kind="Internal")

    with tile.TileContext(nc) as tc, ExitStack() as ctx:
        consts = ctx.enter_context(tc.tile_pool(name="consts", bufs=1))
        sbA = ctx.enter_context(tc.tile_pool(name="sbA", bufs=4))
        psum = ctx.enter_context(tc.tile_pool(name="psum", bufs=3, space="PSUM"))
        psbc = ctx.enter_context(tc.tile_pool(name="psbc", bufs=1, space="PSUM"))
        psad = ctx.enter_context(tc.tile_pool(name="psad", bufs=2, space="PSUM"))
        pst = ctx.enter_context(tc.tile_pool(name="pst", bufs=1, space="PSUM"))
        gpool = ctx.enter_context(tc.tile_pool(name="gpool", bufs=3))
        fpool = ctx.enter_context(tc.tile_pool(name="fpool", bufs=3))
        spool = ctx.enter_context(tc.tile_pool(name="spool", bufs=3))
        ipool = ctx.enter_context(tc.tile_pool(name="ipool", bufs=3))
        epool = ctx.enter_context(tc.tile_pool(name="epool", bufs=3))

        nc.gpsimd.load_library(mlp)

        # ---------------- constants / weight prep ----------------
        iota = consts.tile([P, P], f32)
        nc.sync.dma_start(out=iota[:], in_=IOTA[:])
        iotac = consts.tile([P, 1], f32)
        nc.sync.dma_start(out=iotac[:], in_=IOTAC[:])
        ones1 = consts.tile([1, P], f32)
        nc.vector.memset(ones1[:], 1.0)
        b1t = consts.tile([P, F1], f32)
        nc.sync.dma_start(out=b1t[:], in_=B1[:])
        b2t = consts.tile([P, F2], f32)
        nc.sync.dma_start(out=b2t[:], in_=B2[:])
        ident = consts.tile([P, P], f32)
        make_identity(nc, ident[:])

        rhs1 = consts.tile([P, 198], f32)
        nc.sync.dma_start(out=rhs1[:, :F1], in_=W1[:])
        w1t_a = consts.tile([P, D], f32)
        nc.sync.dma_start(out=w1t_a[:], in_=W1T[0:P, :])
        w1t_b = consts.tile([F1 - P, D], f32)
        nc.sync.dma_start(out=w1t_b[:], in_=W1T[P:F1, :])
        a1_a = consts.tile([P, 6], f32)
        nc.sync.dma_start(out=a1_a[:], in_=A1[0:P, :])
        a1_b = consts.tile([F1 - P, 6], f32)
        nc.sync.dma_start(out=a1_b[:], in_=A1[P:F1, :])
        pu = pst.tile([P, P], f32, tag="tr")
        nc.tensor.matmul(out=pu[:, :6], lhsT=w1t_a[:], rhs=a1_a[:],
                         start=True, stop=False)
        nc.tensor.matmul(out=pu[:, :6], lhsT=w1t_b[:], rhs=a1_b[:],
                         start=False, stop=True)
        nc.vector.tensor_copy(out=rhs1[:, F1:F1 + 6], in_=pu[:, :6])

        w2t = consts.tile([F2, F1], f32)
        nc.sync.dma_start(out=w2t[:], in_=W2T[:])
        a2t = consts.tile([F2, 2], f32)
        nc.sync.dma_start(out=a2t[:], in_=A2[:])
        rhs2_lo = consts.tile([P, 66], f32)
        nc.sync.dma_start(out=rhs2_lo[:, :F2], in_=W2[0:P, :])
        rhs2_hi = consts.tile([F1 - P, 66], f32)
        nc.sync.dma_start(out=rhs2_hi[:, :F2], in_=W2[P:F1, :])
        pu2a = pst.tile([P, P], f32, tag="tr")
        nc.tensor.matmul(out=pu2a[:, :2], lhsT=w2t[:, 0:P], rhs=a2t[:],
                         start=True, stop=True)
        nc.vector.tensor_copy(out=rhs2_lo[:, F2:F2 + 2], in_=pu2a[:, :2])
        pu2b = pst.tile([F1 - P, P], f32, tag="tr2")
        nc.tensor.matmul(out=pu2b[:, :2], lhsT=w2t[:, P:F1], rhs=a2t[:],
                         start=True, stop=True)
        nc.vector.tensor_copy(out=rhs2_hi[:, F2:F2 + 2], in_=pu2b[:, :2])

        # ---------------- stage A ----------------
        for t in range(NT):
            xt = sbA.tile([P, P], f32, tag="xt")
            nc.sync.dma_start(out=xt[:], in_=xT[:, t * P:(t + 1) * P])
            pa = psum.tile([P, 200], f32, tag="mm")
            nc.tensor.matmul(out=pa[:, :198], lhsT=xt[:], rhs=rhs1[:],
                             start=True, stop=True)
            gbf = sbA.tile([P, G1W], bf16, tag="gbf")
            nc.vector.tensor_copy(out=gbf[:, :F1], in_=pa[:, :F1])
            gf32 = gbf[:].bitcast(f32)
            nc.vector.tensor_copy(out=gf32[:, 96:99], in_=pa[:, F1:F1 + 3])
            adw = sbA.tile([P, 4], f32, tag="adw")
            nc.vector.tensor_copy(out=adw[:, :3], in_=pa[:, F1 + 3:F1 + 6])
            if t < HALF // P:
                nc.sync.dma_start(out=G1a[t * P:(t + 1) * P, :], in_=gbf[:])
            else:
                tb = t - HALF // P
                nc.sync.dma_start(out=G1b[tb * P:(tb + 1) * P, :], in_=gbf[:])
            nc.sync.dma_start(out=AD1[t * P:(t + 1) * P, :3], in_=adw[:, :3])

        # ---------------- generic edge phase ----------------
        def edge_layer(LM, TBLa, TBLb, width, nfeat, as_f32col, ADT, adw_,
                       adcol, dreli, drelti, idxwi, blki, ps_width,
                       slot_epilogue):
            nheads = ps_width - nfeat
            NCH = LM["NCH"]
            meta = LM["meta"]
            ops = LM["ops"]
            Ktot = LM["Ktot"]
            blkit = consts.tile([P, NBLK], i32, name=f"blkit{nfeat}")
            nc.sync.dma_start(out=blkit[:], in_=blki[:])
            ps_cur = [None]
            adb_cur = [None]
            fw = nfeat + nheads  # F8 row width

            def new_slot(s):
                adb = epool.tile([P, 4], f32, tag="adb", name="adb")
                nc.gpsimd.indirect_dma_start(
                    out=adb[:, :adw_], out_offset=None, in_=ADT[:],
                    in_offset=IOA(ap=blkit[:, s:s + 1], axis=0))
                adb_cur[0] = adb
                ps_cur[0] = psum.tile([P, 200], f32, tag="mm", name="ps_slot")

            for o, (c0, ncg, tb) in enumerate(ops):
                idxt = ipool.tile([P, GRP * 8], i16, tag="idxt", name="idxt")
                nc.sync.dma_start(
                    out=idxt[:, :ncg * 8],
                    in_=idxwi[:, o * GRP * 8:o * GRP * 8 + ncg * 8])
                drt = ipool.tile([P, GRP], f32, tag="drt", name="drt")
                nc.sync.dma_start(out=drt[:, :ncg],
                                  in_=dreli[:, c0:c0 + ncg])
                grow = gpool.tile([P, GRP, width], bf16, tag="grow",
                                  name="grow")
                nidx = ncg * P
                nc.gpsimd.dma_gather(
                    grow[:, :ncg, :], (TBLb if tb else TBLa)[:],
                    idxt[:, :ncg * 8], nidx, nidx, width)
                # S: [e_part, chunk, d] one-hot
                S8 = spool.tile([P, GRP * P], bf16, tag="s8", name="s8")
                nc.vector.tensor_tensor(
                    out=_ap_view(S8[:], 0, [[P, ncg], [1, P]]),
                    in0=_ap_view(drt[:], 0, [[1, ncg], [0, P]]),
                    in1=_ap_view(iota[:], 0, [[0, ncg], [1, P]]),
                    op=OP.is_equal)
                # S_T + alpha_dst expansion (per SUB-chunk batches)
                adp = psad.tile([P, GRP * nheads], f32, tag="adp", name="adp")
                j = 0
                while j < ncg:
                    sb = min(SUB, ncg - j)
                    drl = ipool.tile([1, SUB * P], f32, tag="drl", name="drl")
                    nc.sync.dma_start(
                        out=drl[:, :sb * P],
                        in_=drelti[:, (c0 + j) * P:(c0 + j + sb) * P])
                    pbc = psbc.tile([P, SUB * P], f32, tag="bc", name="pbc")
                    nc.tensor.matmul(out=pbc[:, :sb * P], lhsT=ones1[:],
                                     rhs=drl[:, :sb * P], start=True, stop=True)
                    st8 = spool.tile([P, SUB * P], f32, tag="st8", name="st8")
                    nc.vector.tensor_scalar(
                        out=st8[:, :sb * P], in0=pbc[:, :sb * P],
                        scalar1=iotac[:, :1], scalar2=None, op0=OP.is_equal)
                    for jj in range(sb):
                        s, k, _tb2 = meta[c0 + j + jj]
                        if k == 0:
                            new_slot(s)
                        nc.tensor.matmul(
                            out=adp[:, (j + jj) * nheads:(j + jj + 1) * nheads],
                            lhsT=st8[:, jj * P:(jj + 1) * P],
                            rhs=adb_cur[0][:, adcol:adcol + nheads],
                            start=True, stop=True)
                    j += sb
                # logits -> exp -> weighted features
                growf = grow[:].bitcast(f32)
                t8 = epool.tile([P, GRP * nheads], f32, tag="t8", name="t8")
                nc.vector.tensor_tensor(
                    out=_ap_view(t8[:], 0, [[nheads, ncg], [1, nheads]]),
                    in0=_ap_view(growf, as_f32col,
                                 [[width // 2, ncg], [1, nheads]]),
                    in1=_ap_view(adp[:], 0, [[nheads, ncg], [1, nheads]]),
                    op=OP.add)
                r8 = epool.tile([P, GRP * nheads], f32, tag="r8", name="r8")
                nc.vector.tensor_scalar(
                    out=r8[:, :ncg * nheads], in0=t8[:, :ncg * nheads],
                    scalar1=0.0, scalar2=SLOPE, op0=OP.min, op1=OP.mult)
                l8 = epool.tile([P, GRP * nheads], f32, tag="l8", name="l8")
                nc.vector.scalar_tensor_tensor(
                    out=l8[:, :ncg * nheads], in0=t8[:, :ncg * nheads],
                    scalar=0.0, in1=r8[:, :ncg * nheads],
                    op0=OP.max, op1=OP.add)
                F8 = fpool.tile([P, GRP * fw], bf16, tag="f8", name="f8")
                nc.scalar.activation(
                    out=_ap_view(F8[:], nfeat, [[fw, ncg], [1, nheads]]),
                    in_=_ap_view(l8[:], 0, [[nheads, ncg], [1, nheads]]),
                    func=AT.Exp)
                hd = nfeat // nheads
                nc.vector.tensor_tensor(
                    out=_ap_view(F8[:], 0, [[fw, ncg], [hd, nheads], [1, hd]]),
                    in0=_ap_view(grow[:], 0,
                                 [[width, ncg], [hd, nheads], [1, hd]]),
                    in1=_ap_view(F8[:], nfeat,
                                 [[fw, ncg], [1, nheads], [0, hd]]),
                    op=OP.mult)
                for jj in range(ncg):
                    s, k, _tb2 = meta[c0 + jj]
                    nc.tensor.matmul(
                        out=ps_cur[0][:, :ps_width],
                        lhsT=S8[:, jj * P:(jj + 1) * P],
                        rhs=F8[:, jj * fw:jj * fw + ps_width],
                        start=(k == 0), stop=(k == Ktot[s] - 1))
                    if k == Ktot[s] - 1:
                        slot_epilogue(s, ps_cur[0])

        # L1 epilogue: h -> transpose -> G2 rows + AD2
        def epi1(s, ps):
            rc = epool.tile([P, H], f32, tag="rc", name="rc")
            nc.vector.tensor_scalar_add(out=rc[:], in0=ps[:, F1:F1 + H],
                                        scalar1=EPS)
            rc2 = epool.tile([P, H], f32, tag="rc2", name="rc2")
            nc.vector.reciprocal(out=rc2[:], in_=rc[:])
            hm = epool.tile([P, F1], f32, tag="hm", name="hm")
            nc.vector.tensor_tensor(
                out=_ap_view(hm[:], 0, [[HID, H], [1, HID]]),
                in0=_ap_view(ps[:, :F1], 0, [[HID, H], [1, HID]]),
                in1=_ap_view(rc2[:], 0, [[1, H], [0, HID]]),
                op=OP.mult)
            hb = epool.tile([P, F1], f32, tag="hb", name="hb")
            nc.vector.tensor_tensor(out=hb[:], in0=hm[:], in1=b1t[:], op=OP.add)
            hr = epool.tile([P, F1], f32, tag="hr", name="hr")
            nc.scalar.activation(out=hr[:], in_=hb[:], func=AT.Relu)
            pt1 = pst.tile([P, P], f32, tag="tr", name="pt1")
            nc.tensor.transpose(out=pt1[:], in_=hr[:, :P], identity=ident[:])
            pt2 = pst.tile([F1 - P, P], f32, tag="tr2", name="pt2")
            nc.tensor.transpose(out=pt2[:], in_=hr[:, P:F1], identity=ident[:])
            ht1 = epool.tile([P, P], f32, tag="ht1", name="ht1")
            nc.vector.tensor_copy(out=ht1[:], in_=pt1[:])
            ht2 = epool.tile([F1 - P, P], f32, tag="ht2", name="ht2")
            nc.vector.tensor_copy(out=ht2[:], in_=pt2[:])
            pg = psum.tile([P, 200], f32, tag="mm", name="pg")
            nc.tensor.matmul(out=pg[:, :66], lhsT=ht1[:], rhs=rhs2_lo[:],
                             start=True, stop=False)
            nc.tensor.matmul(out=pg[:, :66], lhsT=ht2[:], rhs=rhs2_hi[:],
                             start=False, stop=True)
            g2 = epool.tile([P, G2W], bf16, tag="g2", name="g2")
            nc.vector.tensor_copy(out=g2[:, :F2], in_=pg[:, :F2])
            g2f = g2[:].bitcast(f32)
            nc.vector.tensor_copy(out=g2f[:, 32:33], in_=pg[:, F2:F2 + 1])
            ad2w = epool.tile([P, 2], f32, tag="ad2w", name="ad2w")
            nc.vector.tensor_copy(out=ad2w[:, :1], in_=pg[:, F2 + 1:F2 + 2])
            nc.sync.dma_start(out=G2L[s * P:(s + 1) * P, :], in_=g2[:])
            nc.sync.dma_start(out=AD2L[s * P:(s + 1) * P, :1], in_=ad2w[:, :1])

        edge_layer(L1, G1a, G1b, G1W, F1, 96, AD1, 3, 0,
                   DREL1, DRELT1, IDXW1, BLKI, F1 + H, epi1)

        # ---------------- AllGather ----------------
        nc.gpsimd.collective_compute(
            "AllGather", mybir.AluOpType.bypass,
            replica_groups=[list(range(NCORES))],
            ins=[G2L.ap().opt()], outs=[G2F.ap().opt()])
        nc.gpsimd.collective_compute(
            "AllGather", mybir.AluOpType.bypass,
            replica_groups=[list(range(NCORES))],
            ins=[AD2L.ap().opt()], outs=[AD2F.ap().opt()])

        # ---------------- layer 2 ----------------
        def epi2(s, ps):
            rc = epool.tile([P, 1], f32, tag="rcB", name="rcB")
            nc.vector.tensor_scalar_add(out=rc[:], in0=ps[:, F2:F2 + 1],
                                        scalar1=EPS)
            rc2 = epool.tile([P, 1], f32, tag="rcB2", name="rcB2")
            nc.vector.reciprocal(out=rc2[:], in_=rc[:])
            om = epool.tile([P, F2], f32, tag="om", name="om")
            nc.vector.tensor_tensor(out=om[:], in0=ps[:, :F2],
                                    in1=rc2[:].to_broadcast([P, F2]),
                                    op=OP.mult)
            ob = epool.tile([P, F2], f32, tag="ob", name="ob")
            nc.vector.tensor_tensor(out=ob[:], in0=om[:], in1=b2t[:], op=OP.add)
            orl = epool.tile([P, F2], f32, tag="orl", name="orl")
            nc.scalar.activation(out=orl[:], in_=ob[:], func=AT.Relu)
            nc.sync.dma_start(out=OUT[s * P:(s + 1) * P, :], in_=orl[:])

        # G2F views for the two index halves (offsets stay < 2^24 bytes)
        g2fa = G2F[0:HALF, :]
        g2fb = G2F[HALF:NROWC * NCORES, :]
        edge_layer(L2, g2fa, g2fb, G2W, F2, 32, AD2F, 1, 0,
                   DREL2, DRELT2, IDXW2, BLKI2, F2 + 1, epi2)

    nc.compile()
    return nc


def _get_compiled(key, layers):
    if key not in _compiled:
        _compiled[key] = _build(layers[0], layers[1])
    return _compiled[key]


def run(inputs, **runkw):
    from concourse import bass_utils

    key, layers, shared, percore = _host_prep(inputs)
    nc = _get_compiled(key, layers)
    in_maps = []
    for c in range(NCORES):
        m = dict(shared)
        m.update(percore[c])
        in_maps.append(m)
    res = bass_utils.run_bass_kernel_spmd(
        nc, in_maps, core_ids=list(range(NCORES)), **runkw)
    return res


def assemble(results):
    out = np.empty((N, F2), dtype=np.float32)
    for c in range(NCORES):
        out[c * NPC:(c + 1) * NPC] = results[c]["out"][:NPC]
    return out


def kernel(**inputs):
    res = run(inputs)
    return assemble(res.results)


# revision 26
# speedup vs baseline: 1.6677x; 1.1382x over previous
"""GAT 2-layer kernel for Trainium2, 8 NeuronCores (SPMD, dst-sharded).

Strategy (v3):
  - Destination-node sharding: core c owns nodes [c*6250,(c+1)*6250); edges bucketed
    into per-128-dst-node "slots", padded to 128-edge chunks.
  - Stage A (replicated): per 128-node tile one matmul computes
    [x@W1 (192) | alpha_src (3) | alpha_dst (3)]; xw+as go to a bf16 gather table
    G1 (512B rows: 192 bf16 xw + 3 f32 alpha_src bit-packed + pad), ad to slim f32
    table AD1. G1 is split into two <=32768-row tensors (dma_gather int16 index
    limit, 16MiB ucode offset limit).
  - Edge phase per layer: per <=8-chunk group one dma_gather (1024 row gathers/op)
    pulls source rows; one-hot S (DVE is_equal vs iota) segment-reduces
    exp(logit)-weighted features AND the softmax denominators in one per-slot PSUM
    accumulation (normalization pulled out of the sum; logits are O(5), no
    max-subtraction needed). alpha_dst[dst] is expanded edge-wise on the PE:
    one-hot-transpose S_T (built by K=1 ones-matmul broadcast of dst_rel + DVE
    compare) times the slot's alpha_dst block (gathered once per slot).
  - Per-slot epilogue: h = relu(sum/(denom+eps) + bias1); PE-transpose h and
    immediately emit G2 rows [h@W2 (64) bf16 | as2 f32] and slim AD2; AllGather
    both; layer 2 repeats the edge phase (1 head) against G2F views.
"""
import sys

sys.path.insert(0, "/opt/trn_rl_repo")
import numpy as np
import ml_dtypes

N = 50000
D = 128
HID = 64
H = 3
F1 = 192
F2 = 64
NCORES = 8
NPC = N // NCORES          # 6250 nodes per core
P = 128
NBLK = (NPC + P - 1) // P  # 49 slots per core
NT = (N + P - 1) // P      # 391 stage-A node tiles
NROW1 = NT * P             # 50048 G1 rows
HALF = 32768               # dma_gather int16 index limit
G1W = 256                  # bf16 cols: xw(192) | as f32 x3 (bf16 192:198) | pad
G2W = 128                  # bf16 cols: xw2(64) | as2 f32 (bf16 64:66) | pad
NROWC = NBLK * P           # 6272 rows per core shard
SLOPE = 0.2
EPS = 1e-16
GRP = 8                    # max chunks per dma_gather / op group
SUB = 4                    # chunks per S_T broadcast matmul (512 psum cols)

_compiled = {}


def _chunkize(src_key, dst, order_all):
    """Bucket edges per (core, slot), split by src_key half, pad to 128.
    Returns per-core flat arrays + compile-time chunk structure (shared)."""
    core = dst // NPC
    rel = dst % NPC
    slot = rel // P
    half = (src_key >= HALF).astype(np.int64)
    # counts[core, slot, half]
    counts = np.zeros((NCORES, NBLK, 2), dtype=np.int64)
    np.add.at(counts, (core, slot, half), 1)
    Ka = np.ceil(counts[:, :, 0] / P).astype(np.int64).max(axis=0)
    Kb = np.ceil(counts[:, :, 1] / P).astype(np.int64).max(axis=0)
    return Ka, Kb


def _host_prep(inputs):
    x = np.asarray(inputs["x"], dtype=np.float32)
    ei = np.asarray(inputs["edge_index"])
    W1 = np.asarray(inputs["W1"], dtype=np.float32)
    as1 = np.asarray(inputs["att_src1"], dtype=np.float32)
    ad1 = np.asarray(inputs["att_dst1"], dtype=np.float32)
    b1 = np.asarray(inputs["bias1"], dtype=np.float32)
    W2 = np.asarray(inputs["W2"], dtype=np.float32)
    as2 = np.asarray(inputs["att_src2"], dtype=np.float32)
    ad2 = np.asarray(inputs["att_dst2"], dtype=np.float32)
    b2 = np.asarray(inputs["bias2"], dtype=np.float32)

    loops = np.arange(N, dtype=np.int64)
    src = np.concatenate([ei[0].astype(np.int64), loops])
    dst = np.concatenate([ei[1].astype(np.int64), loops])
    order = np.argsort(dst, kind="stable")
    src = src[order]
    dst = dst[order]
    g2row = (src // NPC) * NROWC + (src % NPC)

    # chunk structure per layer (uniform across cores)
    Ka1, Kb1 = _chunkize(src, dst, None)
    Ka2, Kb2 = _chunkize(g2row, dst, None)

    def build_layer(key):
        Ka, Kb = (Ka1, Kb1) if key == 1 else (Ka2, Kb2)
        skey = src if key == 1 else g2row
        NCH = int((Ka + Kb).sum())
        # chunk meta: (slot, k_in_slot, table) in processing order
        meta = []
        for s in range(NBLK):
            k = 0
            for _ in range(int(Ka[s])):
                meta.append((s, k, 0)); k += 1
            for _ in range(int(Kb[s])):
                meta.append((s, k, 1)); k += 1
        # gather ops: runs of <=GRP same-table consecutive chunks
        ops = []   # (chunk_start, n_chunks, table)
        i = 0
        while i < NCH:
            t = meta[i][2]
            j = i
            while j < NCH and j - i < GRP and meta[j][2] == t:
                j += 1
            ops.append((i, j - i, t))
            i = j
        NOPS = len(ops)

        EPAD = NCH * P
        SRCK = np.zeros((NCORES, EPAD), dtype=np.int64)
        DREL = np.full((NCORES, EPAD), 255.0, dtype=np.float32)
        for c in range(NCORES):
            base_node = c * NPC
            # per-slot edge ranges (dst-sorted => contiguous)
            cb = 0
            for s in range(NBLK):
                blo = base_node + s * P
                bhi = min(blo + P, base_node + NPC)
                lo = np.searchsorted(dst, blo, side="left")
                hi = np.searchsorted(dst, bhi, side="left")
                sk = skey[lo:hi]
                dr = (dst[lo:hi] - blo).astype(np.float32)
                a_mask = sk < HALF
                for which, KK, pad in ((a_mask, Ka[s], 0),
                                       (~a_mask, Kb[s], HALF)):
                    cnt = int(which.sum())
                    pos = cb * P
                    SRCK[c, pos:pos + cnt] = sk[which]
                    # pad indices must stay valid for the table half
                    SRCK[c, pos + cnt:(cb + int(KK)) * P] = pad
                    DREL[c, pos:pos + cnt] = dr[which]
                    cb += int(KK)
        # device arrays
        DREL_t = np.ascontiguousarray(
            DREL.reshape(NCORES, NCH, P).transpose(0, 2, 1))
        DRELT = np.ascontiguousarray(DREL.reshape(NCORES, 1, EPAD))
        # wrapped int16 indices per gather op, [128, NOPS*64]
        IDXW = np.zeros((NCORES, P, NOPS * GRP * 8), dtype=np.int16)
        for c in range(NCORES):
            for o, (c0, ncg, t) in enumerate(ops):
                iv = SRCK[c, c0 * P:(c0 + ncg) * P] - (HALF if t else 0)
                w = iv.reshape(-1, 16).T.astype(np.int16)  # [16, n/16]
                IDXW[c, :, o * GRP * 8: o * GRP * 8 + w.shape[1]] = \
                    np.tile(w, (8, 1))
        return dict(NCH=NCH, meta=meta, ops=ops, NOPS=NOPS,
                    Ktot=[int(Ka[s] + Kb[s]) for s in range(NBLK)],
                    DREL=DREL_t, DRELT=DRELT, IDXW=IDXW)

    L1 = build_layer(1)
    L2 = build_layer(2)

    # per-slot block-node gather indices (alpha_dst blocks)
    BLKI = np.zeros((NCORES, P, NBLK), dtype=np.int32)
    BLKI2 = np.zeros((NCORES, P, NBLK), dtype=np.int32)
    for c in range(NCORES):
        for s in range(NBLK):
            nodes = np.minimum(c * NPC + s * P + np.arange(P), N - 1)
            BLKI[c, :, s] = nodes
            BLKI2[c, :, s] = (nodes // NPC) * NROWC + (nodes % NPC)

    xT = np.zeros((D, NROW1), dtype=np.float32)
    xT[:, :N] = x.T
    A1 = np.zeros((F1, 6), dtype=np.float32)
    for h in range(H):
        A1[h * HID:(h + 1) * HID, h] = as1[h]
        A1[h * HID:(h + 1) * HID, 3 + h] = ad1[h]
    A2 = np.stack([as2[0], ad2[0]], axis=1).astype(np.float32)

    shared = {
        "xT": xT,
        "W1": np.ascontiguousarray(W1),
        "W1T": np.ascontiguousarray(W1.T),
        "A1": A1,
        "W2": np.ascontiguousarray(W2),
        "W2T": np.ascontiguousarray(W2.T),
        "A2": A2,
        "B1": np.ascontiguousarray(np.broadcast_to(b1, (P, F1))),
        "B2": np.ascontiguousarray(np.broadcast_to(b2, (P, F2))),
        "IOTA": np.ascontiguousarray(
            np.broadcast_to(np.arange(P, dtype=np.float32), (P, P))),
        "IOTAC": np.arange(P, dtype=np.float32).reshape(P, 1),
    }
    percore = []
    for c in range(NCORES):
        percore.append({
            "DREL1": L1["DREL"][c], "DRELT1": L1["DRELT"][c],
            "IDXW1": L1["IDXW"][c],
            "DREL2": L2["DREL"][c], "DRELT2": L2["DRELT"][c],
            "IDXW2": L2["IDXW"][c],
            "BLKI": BLKI[c], "BLKI2": BLKI2[c],
        })
    key = (tuple(L1["Ktot"]), tuple(x[0] for x in L1["ops"]),
           tuple(x[1] for x in L1["ops"]), tuple(x[2] for x in L1["ops"]),
           tuple(L2["Ktot"]), tuple(x[0] for x in L2["ops"]),
           tuple(x[1] for x in L2["ops"]), tuple(x[2] for x in L2["ops"]))
    return key, (L1, L2), shared, percore


def _ap_view(ap, extra_offset, free_dims):
    import concourse.bass as bass

    return bass.AP(
        tensor=ap.tensor, offset=ap.offset + extra_offset,
        ap=[list(ap.ap[0])] + [list(d) for d in free_dims],
    )


def _build(L1, L2):
    import concourse.bass as bass
    import concourse.bacc as bacc
    import concourse.tile as tile
    from concourse import mybir
    from concourse.masks import make_identity
    from concourse.library_config import mlp
    from contextlib import ExitStack

    f32 = mybir.dt.float32
    bf16 = mybir.dt.bfloat16
    i32 = mybir.dt.int32
    i16 = mybir.dt.int16
    AT = mybir.ActivationFunctionType
    OP = mybir.AluOpType
    IOA = bass.IndirectOffsetOnAxis

    nc = bacc.Bacc("TRN2", target_bir_lowering=False, debug=False,
                   num_devices=NCORES)

    xT = nc.dram_tensor("xT", [D, NROW1], f32, kind="ExternalInput")
    W1 = nc.dram_tensor("W1", [D, F1], f32, kind="ExternalInput")
    W1T = nc.dram_tensor("W1T", [F1, D], f32, kind="ExternalInput")
    A1 = nc.dram_tensor("A1", [F1, 6], f32, kind="ExternalInput")
    W2 = nc.dram_tensor("W2", [F1, F2], f32, kind="ExternalInput")
    W2T = nc.dram_tensor("W2T", [F2, F1], f32, kind="ExternalInput")
    A2 = nc.dram_tensor("A2", [F2, 2], f32, kind="ExternalInput")
    B1 = nc.dram_tensor("B1", [P, F1], f32, kind="ExternalInput")
    B2 = nc.dram_tensor("B2", [P, F2], f32, kind="ExternalInput")
    IOTA = nc.dram_tensor("IOTA", [P, P], f32, kind="ExternalInput")
    IOTAC = nc.dram_tensor("IOTAC", [P, 1], f32, kind="ExternalInput")
    DREL1 = nc.dram_tensor("DREL1", [P, L1["NCH"]], f32, kind="ExternalInput")
    DRELT1 = nc.dram_tensor("DRELT1", [1, L1["NCH"] * P], f32,
                            kind="ExternalInput")
    IDXW1 = nc.dram_tensor("IDXW1", [P, L1["NOPS"] * GRP * 8], i16,
                           kind="ExternalInput")
    DREL2 = nc.dram_tensor("DREL2", [P, L2["NCH"]], f32, kind="ExternalInput")
    DRELT2 = nc.dram_tensor("DRELT2", [1, L2["NCH"] * P], f32,
                            kind="ExternalInput")
    IDXW2 = nc.dram_tensor("IDXW2", [P, L2["NOPS"] * GRP * 8], i16,
                           kind="ExternalInput")
    BLKI = nc.dram_tensor("BLKI", [P, NBLK], i32, kind="ExternalInput")
    BLKI2 = nc.dram_tensor("BLKI2", [P, NBLK], i32, kind="ExternalInput")
    OUT = nc.dram_tensor("out", [NROWC, F2], f32, kind="ExternalOutput")

    G1a = nc.dram_tensor("G1a", [HALF, G1W], bf16, kind="Internal")
    G1b = nc.dram_tensor("G1b", [NROW1 - HALF, G1W], bf16, kind="Internal")
    AD1 = nc.dram_tensor("AD1", [NROW1, 4], f32, kind="Internal")
    G2L = nc.dram_tensor("G2L", [NROWC, G2W], bf16, kind="Internal")
    AD2L = nc.dram_tensor("AD2L", [NROWC, 2], f32, kind="Internal")
    G2F = nc.dram_tensor("G2F", [NROWC * NCORES, G2W], bf16,
                         addr_space="Shared", kind="Internal")
    AD2F = nc.dram_tensor("AD2F", [NROWC * NCORES, 2], f32,
                          addr_space="Shared", kind="Internal")

    with tile.TileContext(nc) as tc, ExitStack() as ctx:
        consts = ctx.enter_context(tc.tile_pool(name="consts", bufs=1))
        sbA = ctx.enter_context(tc.tile_pool(name="sbA", bufs=6))
        psum = ctx.enter_context(tc.tile_pool(name="psum", bufs=3, space="PSUM"))
        psbc = ctx.enter_context(tc.tile_pool(name="psbc", bufs=1, space="PSUM"))
        psad = ctx.enter_context(tc.tile_pool(name="psad", bufs=2, space="PSUM"))
        pst = ctx.enter_context(tc.tile_pool(name="pst", bufs=1, space="PSUM"))
        gpool = ctx.enter_context(tc.tile_pool(name="gpool", bufs=4))
        fpool = ctx.enter_context(tc.tile_pool(name="fpool", bufs=4))
        spool = ctx.enter_context(tc.tile_pool(name="spool", bufs=4))
        ipool = ctx.enter_context(tc.tile_pool(name="ipool", bufs=4))
        epool = ctx.enter_context(tc.tile_pool(name="epool", bufs=3))

        nc.gpsimd.load_library(mlp)

        # ---------------- constants / weight prep ----------------
        iota = consts.tile([P, P], f32)
        nc.sync.dma_start(out=iota[:], in_=IOTA[:])
        iotac = consts.tile([P, 1], f32)
        nc.sync.dma_start(out=iotac[:], in_=IOTAC[:])
        ones1 = consts.tile([1, P], f32)
        nc.vector.memset(ones1[:], 1.0)
        b1t = consts.tile([P, F1], f32)
        nc.sync.dma_start(out=b1t[:], in_=B1[:])
        b2t = consts.tile([P, F2], f32)
        nc.sync.dma_start(out=b2t[:], in_=B2[:])
        ident = consts.tile([P, P], f32)
        make_identity(nc, ident[:])

        rhs1 = consts.tile([P, 198], f32)
        nc.sync.dma_start(out=rhs1[:, :F1], in_=W1[:])
        w1t_a = consts.tile([P, D], f32)
        nc.sync.dma_start(out=w1t_a[:], in_=W1T[0:P, :])
        w1t_b = consts.tile([F1 - P, D], f32)
        nc.sync.dma_start(out=w1t_b[:], in_=W1T[P:F1, :])
        a1_a = consts.tile([P, 6], f32)
        nc.sync.dma_start(out=a1_a[:], in_=A1[0:P, :])
        a1_b = consts.tile([F1 - P, 6], f32)
        nc.sync.dma_start(out=a1_b[:], in_=A1[P:F1, :])
        pu = pst.tile([P, P], f32, tag="tr")
        nc.tensor.matmul(out=pu[:, :6], lhsT=w1t_a[:], rhs=a1_a[:],
                         start=True, stop=False)
        nc.tensor.matmul(out=pu[:, :6], lhsT=w1t_b[:], rhs=a1_b[:],
                         start=False, stop=True)
        nc.vector.tensor_copy(out=rhs1[:, F1:F1 + 6], in_=pu[:, :6])

        w2t = consts.tile([F2, F1], f32)
        nc.sync.dma_start(out=w2t[:], in_=W2T[:])
        a2t = consts.tile([F2, 2], f32)
        nc.sync.dma_start(out=a2t[:], in_=A2[:])
        rhs2_lo = consts.tile([P, 66], f32)
        nc.sync.dma_start(out=rhs2_lo[:, :F2], in_=W2[0:P, :])
        rhs2_hi = consts.tile([F1 - P, 66], f32)
        nc.sync.dma_start(out=rhs2_hi[:, :F2], in_=W2[P:F1, :])
        pu2a = pst.tile([P, P], f32, tag="tr")
        nc.tensor.matmul(out=pu2a[:, :2], lhsT=w2t[:, 0:P], rhs=a2t[:],
                         start=True, stop=True)
        nc.vector.tensor_copy(out=rhs2_lo[:, F2:F2 + 2], in_=pu2a[:, :2])
        pu2b = pst.tile([F1 - P, P], f32, tag="tr2")
        nc.tensor.matmul(out=pu2b[:, :2], lhsT=w2t[:, P:F1], rhs=a2t[:],
                         start=True, stop=True)
        nc.vector.tensor_copy(out=rhs2_hi[:, F2:F2 + 2], in_=pu2b[:, :2])

        # ---------------- stage A ----------------
        for t in range(NT):
            xt = sbA.tile([P, P], f32, tag="xt")
            nc.sync.dma_start(out=xt[:], in_=xT[:, t * P:(t + 1) * P])
            pa = psum.tile([P, 200], f32, tag="mm")
            nc.tensor.matmul(out=pa[:, :198], lhsT=xt[:], rhs=rhs1[:],
                             start=True, stop=True)
            gbf = sbA.tile([P, G1W], bf16, tag="gbf")
            nc.vector.tensor_copy(out=gbf[:, :F1], in_=pa[:, :F1])
            gf32 = gbf[:].bitcast(f32)
            nc.vector.tensor_copy(out=gf32[:, 96:99], in_=pa[:, F1:F1 + 3])
            adw = sbA.tile([P, 4], f32, tag="adw")
            nc.vector.tensor_copy(out=adw[:, :3], in_=pa[:, F1 + 3:F1 + 6])
            if t < HALF // P:
                nc.scalar.dma_start(out=G1a[t * P:(t + 1) * P, :], in_=gbf[:])
            else:
                tb = t - HALF // P
                nc.scalar.dma_start(out=G1b[tb * P:(tb + 1) * P, :], in_=gbf[:])
            nc.scalar.dma_start(out=AD1[t * P:(t + 1) * P, :3], in_=adw[:, :3])

        # ---------------- generic edge phase ----------------
        def edge_layer(LM, TBLa, TBLb, width, nfeat, as_f32col, ADT, adw_,
                       adcol, dreli, drelti, idxwi, blki, ps_width,
                       slot_epilogue):
            nheads = ps_width - nfeat
            NCH = LM["NCH"]
            meta = LM["meta"]
            ops = LM["ops"]
            Ktot = LM["Ktot"]
            blkit = consts.tile([P, NBLK], i32, name=f"blkit{nfeat}")
            nc.sync.dma_start(out=blkit[:], in_=blki[:])
            ps_cur = [None]
            adb_cur = [None]
            fw = nfeat + nheads  # F8 row width

            def new_slot(s):
                adb = epool.tile([P, 4], f32, tag="adb", name="adb")
                nc.gpsimd.indirect_dma_start(
                    out=adb[:, :adw_], out_offset=None, in_=ADT[:],
                    in_offset=IOA(ap=blkit[:, s:s + 1], axis=0))
                adbh = epool.tile([P, 4], bf16, tag="adbh", name="adbh")
                nc.vector.tensor_copy(out=adbh[:, :adw_], in_=adb[:, :adw_])
                adb_cur[0] = adbh
                ps_cur[0] = psum.tile([P, 200], f32, tag="mm", name="ps_slot")

            for o, (c0, ncg, tb) in enumerate(ops):
                idxt = ipool.tile([P, GRP * 8], i16, tag="idxt", name="idxt")
                nc.sync.dma_start(
                    out=idxt[:, :ncg * 8],
                    in_=idxwi[:, o * GRP * 8:o * GRP * 8 + ncg * 8])
                drt = ipool.tile([P, GRP], f32, tag="drt", name="drt")
                nc.sync.dma_start(out=drt[:, :ncg],
                                  in_=dreli[:, c0:c0 + ncg])
                grow = gpool.tile([P, GRP, width], bf16, tag="grow",
                                  name="grow")
                nidx = ncg * P
                nc.gpsimd.dma_gather(
                    grow[:, :ncg, :], (TBLb if tb else TBLa)[:],
                    idxt[:, :ncg * 8], nidx, nidx, width)
                # S: [e_part, chunk, d] one-hot
                S8 = spool.tile([P, GRP * P], bf16, tag="s8", name="s8")
                nc.vector.tensor_tensor(
                    out=_ap_view(S8[:], 0, [[P, ncg], [1, P]]),
                    in0=_ap_view(drt[:], 0, [[1, ncg], [0, P]]),
                    in1=_ap_view(iota[:], 0, [[0, ncg], [1, P]]),
                    op=OP.is_equal)
                # S_T + alpha_dst expansion (per SUB-chunk batches)
                adp = psad.tile([P, GRP * nheads], f32, tag="adp", name="adp")
                j = 0
                while j < ncg:
                    sb = min(SUB, ncg - j)
                    drl = ipool.tile([1, SUB * P], f32, tag="drl", name="drl")
                    nc.scalar.dma_start(
                        out=drl[:, :sb * P],
                        in_=drelti[:, (c0 + j) * P:(c0 + j + sb) * P])
                    pbc = psbc.tile([P, SUB * P], f32, tag="bc", name="pbc")
                    nc.tensor.matmul(out=pbc[:, :sb * P], lhsT=ones1[:],
                                     rhs=drl[:, :sb * P], start=True, stop=True)
                    st8 = spool.tile([P, SUB * P], bf16, tag="st8", name="st8")
                    nc.vector.tensor_scalar(
                        out=st8[:, :sb * P], in0=pbc[:, :sb * P],
                        scalar1=iotac[:, :1], scalar2=None, op0=OP.is_equal)
                    for jj in range(sb):
                        s, k, _tb2 = meta[c0 + j + jj]
                        if k == 0:
                            new_slot(s)
                        nc.tensor.matmul(
                            out=adp[:, (j + jj) * nheads:(j + jj + 1) * nheads],
                            lhsT=st8[:, jj * P:(jj + 1) * P],
                            rhs=adb_cur[0][:, adcol:adcol + nheads],
                            start=True, stop=True)
                    j += sb
                # logits -> exp -> weighted features
                growf = grow[:].bitcast(f32)
                t8 = epool.tile([P, GRP * nheads], f32, tag="t8", name="t8")
                nc.vector.tensor_tensor(
                    out=_ap_view(t8[:], 0, [[nheads, ncg], [1, nheads]]),
                    in0=_ap_view(growf, as_f32col,
                                 [[width // 2, ncg], [1, nheads]]),
                    in1=_ap_view(adp[:], 0, [[nheads, ncg], [1, nheads]]),
                    op=OP.add)
                r8 = epool.tile([P, GRP * nheads], f32, tag="r8", name="r8")
                nc.vector.tensor_scalar(
                    out=r8[:, :ncg * nheads], in0=t8[:, :ncg * nheads],
                    scalar1=0.0, scalar2=SLOPE, op0=OP.min, op1=OP.mult)
                l8 = epool.tile([P, GRP * nheads], f32, tag="l8", name="l8")
                nc.vector.scalar_tensor_tensor(
                    out=l8[:, :ncg * nheads], in0=t8[:, :ncg * nheads],
                    scalar=0.0, in1=r8[:, :ncg * nheads],
                    op0=OP.max, op1=OP.add)
                F8 = fpool.tile([P, GRP * fw], bf16, tag="f8", name="f8")
                nc.scalar.activation(
                    out=_ap_view(F8[:], nfeat, [[fw, ncg], [1, nheads]]),
                    in_=_ap_view(l8[:], 0, [[nheads, ncg], [1, nheads]]),
                    func=AT.Exp)
                hd = nfeat // nheads
                nc.vector.tensor_tensor(
                    out=_ap_view(F8[:], 0, [[fw, ncg], [hd, nheads], [1, hd]]),
                    in0=_ap_view(grow[:], 0,
                                 [[width, ncg], [hd, nheads], [1, hd]]),
                    in1=_ap_view(F8[:], nfeat,
                                 [[fw, ncg], [1, nheads], [0, hd]]),
                    op=OP.mult)
                for jj in range(ncg):
                    s, k, _tb2 = meta[c0 + jj]
                    nc.tensor.matmul(
                        out=ps_cur[0][:, :ps_width],
                        lhsT=S8[:, jj * P:(jj + 1) * P],
                        rhs=F8[:, jj * fw:jj * fw + ps_width],
                        start=(k == 0), stop=(k == Ktot[s] - 1))
                    if k == Ktot[s] - 1:
                        slot_epilogue(s, ps_cur[0])

        # L1 epilogue: h -> transpose -> G2 rows + AD2
        def epi1(s, ps):
            rc = epool.tile([P, H], f32, tag="rc", name="rc")
            nc.vector.tensor_scalar_add(out=rc[:], in0=ps[:, F1:F1 + H],
                                        scalar1=EPS)
            rc2 = epool.tile([P, H], f32, tag="rc2", name="rc2")
            nc.vector.reciprocal(out=rc2[:], in_=rc[:])
            hm = epool.tile([P, F1], f32, tag="hm", name="hm")
            nc.vector.tensor_tensor(
                out=_ap_view(hm[:], 0, [[HID, H], [1, HID]]),
                in0=_ap_view(ps[:, :F1], 0, [[HID, H], [1, HID]]),
                in1=_ap_view(rc2[:], 0, [[1, H], [0, HID]]),
                op=OP.mult)
            hb = epool.tile([P, F1], f32, tag="hb", name="hb")
            nc.vector.tensor_tensor(out=hb[:], in0=hm[:], in1=b1t[:], op=OP.add)
            hr = epool.tile([P, F1], f32, tag="hr", name="hr")
            nc.scalar.activation(out=hr[:], in_=hb[:], func=AT.Relu)
            pt1 = pst.tile([P, P], f32, tag="tr", name="pt1")
            nc.tensor.transpose(out=pt1[:], in_=hr[:, :P], identity=ident[:])
            pt2 = pst.tile([F1 - P, P], f32, tag="tr2", name="pt2")
            nc.tensor.transpose(out=pt2[:], in_=hr[:, P:F1], identity=ident[:])
            ht1 = epool.tile([P, P], f32, tag="ht1", name="ht1")
            nc.vector.tensor_copy(out=ht1[:], in_=pt1[:])
            ht2 = epool.tile([F1 - P, P], f32, tag="ht2", name="ht2")
            nc.vector.tensor_copy(out=ht2[:], in_=pt2[:])
            pg = psum.tile([P, 200], f32, tag="mm", name="pg")
            nc.tensor.matmul(out=pg[:, :66], lhsT=ht1[:], rhs=rhs2_lo[:],
                             start=True, stop=False)
            nc.tensor.matmul(out=pg[:, :66], lhsT=ht2[:], rhs=rhs2_hi[:],
                             start=False, stop=True)
            g2 = epool.tile([P, G2W], bf16, tag="g2", name="g2")
            nc.vector.tensor_copy(out=g2[:, :F2], in_=pg[:, :F2])
            g2f = g2[:].bitcast(f32)
            nc.vector.tensor_copy(out=g2f[:, 32:33], in_=pg[:, F2:F2 + 1])
            ad2w = epool.tile([P, 2], f32, tag="ad2w", name="ad2w")
            nc.vector.tensor_copy(out=ad2w[:, :1], in_=pg[:, F2 + 1:F2 + 2])
            nc.sync.dma_start(out=G2L[s * P:(s + 1) * P, :], in_=g2[:])
            nc.sync.dma_start(out=AD2L[s * P:(s + 1) * P, :1], in_=ad2w[:, :1])

        edge_layer(L1, G1a, G1b, G1W, F1, 96, AD1, 3, 0,
                   DREL1, DRELT1, IDXW1, BLKI, F1 + H, epi1)

        # ---------------- AllGather ----------------
        nc.gpsimd.collective_compute(
            "AllGather", mybir.AluOpType.bypass,
            replica_groups=[list(range(NCORES))],
            ins=[G2L.ap().opt()], outs=[G2F.ap().opt()])
        nc.gpsimd.collective_compute(
            "AllGather", mybir.AluOpType.bypass,
            replica_groups=[list(range(NCORES))],
            ins=[AD2L.ap().opt()], outs=[AD2F.ap().opt()])

        # ---------------- layer 2 ----------------
        def epi2(s, ps):
            rc = epool.tile([P, 1], f32, tag="rcB", name="rcB")
            nc.vector.tensor_scalar_add(out=rc[:], in0=ps[:, F2:F2 + 1],
                                        scalar1=EPS)
            rc2 = epool.tile([P, 1], f32, tag="rcB2", name="rcB2")
            nc.vector.reciprocal(out=rc2[:], in_=rc[:])
            om = epool.tile([P, F2], f32, tag="om", name="om")
            nc.vector.tensor_tensor(out=om[:], in0=ps[:, :F2],
                                    in1=rc2[:].to_broadcast([P, F2]),
                                    op=OP.mult)
            ob = epool.tile([P, F2], f32, tag="ob", name="ob")
            nc.vector.tensor_tensor(out=ob[:], in0=om[:], in1=b2t[:], op=OP.add)
            orl = epool.tile([P, F2], f32, tag="orl", name="orl")
            nc.scalar.activation(out=orl[:], in_=ob[:], func=AT.Relu)
            nc.sync.dma_start(out=OUT[s * P:(s + 1) * P, :], in_=orl[:])

        # G2F views for the two index halves (offsets stay < 2^24 bytes)
        g2fa = G2F[0:HALF, :]
        g2fb = G2F[HALF:NROWC * NCORES, :]
        edge_layer(L2, g2fa, g2fb, G2W, F2, 32, AD2F, 1, 0,
                   DREL2, DRELT2, IDXW2, BLKI2, F2 + 1, epi2)

    nc.compile()
    return nc


def _get_compiled(key, layers):
    if key not in _compiled:
        _compiled[key] = _build(layers[0], layers[1])
    return _compiled[key]


def run(inputs, **runkw):
    from concourse import bass_utils

    key, layers, shared, percore = _host_prep(inputs)
    nc = _get_compiled(key, layers)
    in_maps = []
    for c in range(NCORES):
        m = dict(shared)
        m.update(percore[c])
        in_maps.append(m)
    res = bass_utils.run_bass_kernel_spmd(
        nc, in_maps, core_ids=list(range(NCORES)), **runkw)
    return res


def assemble(results):
    out = np.empty((N, F2), dtype=np.float32)
    for c in range(NCORES):
        out[c * NPC:(c + 1) * NPC] = results[c]["out"][:NPC]
    return out


def kernel(**inputs):
    res = run(inputs)
    return assemble(res.results)


# revision 28
# speedup vs baseline: 1.7280x; 1.0362x over previous
"""GAT 2-layer kernel for Trainium2, 8 NeuronCores (SPMD, dst-sharded).

Strategy (v3):
  - Destination-node sharding: core c owns nodes [c*6250,(c+1)*6250); edges bucketed
    into per-128-dst-node "slots", padded to 128-edge chunks.
  - Stage A (replicated): per 128-node tile one matmul computes
    [x@W1 (192) | alpha_src (3) | alpha_dst (3)]; xw+as go to a bf16 gather table
    G1 (512B rows: 192 bf16 xw + 3 f32 alpha_src bit-packed + pad), ad to slim f32
    table AD1. G1 is split into two <=32768-row tensors (dma_gather int16 index
    limit, 16MiB ucode offset limit).
  - Edge phase per layer: per <=8-chunk group one dma_gather (1024 row gathers/op)
    pulls source rows; one-hot S (DVE is_equal vs iota) segment-reduces
    exp(logit)-weighted features AND the softmax denominators in one per-slot PSUM
    accumulation (normalization pulled out of the sum; logits are O(5), no
    max-subtraction needed). alpha_dst[dst] is expanded edge-wise on the PE:
    one-hot-transpose S_T (built by K=1 ones-matmul broadcast of dst_rel + DVE
    compare) times the slot's alpha_dst block (gathered once per slot).
  - Per-slot epilogue: h = relu(sum/(denom+eps) + bias1); PE-transpose h and
    immediately emit G2 rows [h@W2 (64) bf16 | as2 f32] and slim AD2; AllGather
    both; layer 2 repeats the edge phase (1 head) against G2F views.
"""
import sys

sys.path.insert(0, "/opt/trn_rl_repo")
import numpy as np
import ml_dtypes

N = 50000
D = 128
HID = 64
H = 3
F1 = 192
F2 = 64
NCORES = 8
NPC = N // NCORES          # 6250 nodes per core
P = 128
NBLK = (NPC + P - 1) // P  # 49 slots per core
NT = (N + P - 1) // P      # 391 stage-A node tiles
NROW1 = NT * P             # 50048 G1 rows
HALF = 32768               # dma_gather int16 index limit
G1W = 256                  # bf16 cols: xw(192) | as f32 x3 (bf16 192:198) | pad
G2W = 128                  # bf16 cols: xw2(64) | as2 f32 (bf16 64:66) | pad
NROWC = NBLK * P           # 6272 rows per core shard
SLOPE = 0.2
EPS = 1e-16
GRP = 8                    # max chunks per dma_gather / op group
SUB = 4                    # chunks per S_T broadcast matmul (512 psum cols)

_compiled = {}


def _chunkize(src_key, dst, order_all):
    """Bucket edges per (core, slot), split by src_key half, pad to 128.
    Returns per-core flat arrays + compile-time chunk structure (shared)."""
    core = dst // NPC
    rel = dst % NPC
    slot = rel // P
    half = (src_key >= HALF).astype(np.int64)
    # counts[core, slot, half]
    counts = np.zeros((NCORES, NBLK, 2), dtype=np.int64)
    np.add.at(counts, (core, slot, half), 1)
    Ka = np.ceil(counts[:, :, 0] / P).astype(np.int64).max(axis=0)
    Kb = np.ceil(counts[:, :, 1] / P).astype(np.int64).max(axis=0)
    return Ka, Kb


def _host_prep(inputs):
    x = np.asarray(inputs["x"], dtype=np.float32)
    ei = np.asarray(inputs["edge_index"])
    W1 = np.asarray(inputs["W1"], dtype=np.float32)
    as1 = np.asarray(inputs["att_src1"], dtype=np.float32)
    ad1 = np.asarray(inputs["att_dst1"], dtype=np.float32)
    b1 = np.asarray(inputs["bias1"], dtype=np.float32)
    W2 = np.asarray(inputs["W2"], dtype=np.float32)
    as2 = np.asarray(inputs["att_src2"], dtype=np.float32)
    ad2 = np.asarray(inputs["att_dst2"], dtype=np.float32)
    b2 = np.asarray(inputs["bias2"], dtype=np.float32)

    loops = np.arange(N, dtype=np.int64)
    src = np.concatenate([ei[0].astype(np.int64), loops])
    dst = np.concatenate([ei[1].astype(np.int64), loops])
    order = np.argsort(dst, kind="stable")
    src = src[order]
    dst = dst[order]
    g2row = (src // NPC) * NROWC + (src % NPC)

    # chunk structure per layer (uniform across cores)
    Ka1, Kb1 = _chunkize(src, dst, None)
    Ka2, Kb2 = _chunkize(g2row, dst, None)

    def build_layer(key):
        Ka, Kb = (Ka1, Kb1) if key == 1 else (Ka2, Kb2)
        skey = src if key == 1 else g2row
        NCH = int((Ka + Kb).sum())
        # chunk meta: (slot, k_in_slot, table) in processing order
        meta = []
        for s in range(NBLK):
            k = 0
            for _ in range(int(Ka[s])):
                meta.append((s, k, 0)); k += 1
            for _ in range(int(Kb[s])):
                meta.append((s, k, 1)); k += 1
        # gather ops: runs of <=GRP same-table consecutive chunks
        ops = []   # (chunk_start, n_chunks, table)
        i = 0
        while i < NCH:
            t = meta[i][2]
            j = i
            while j < NCH and j - i < GRP and meta[j][2] == t:
                j += 1
            ops.append((i, j - i, t))
            i = j
        NOPS = len(ops)

        EPAD = NCH * P
        SRCK = np.zeros((NCORES, EPAD), dtype=np.int64)
        DREL = np.full((NCORES, EPAD), 255.0, dtype=np.float32)
        for c in range(NCORES):
            base_node = c * NPC
            # per-slot edge ranges (dst-sorted => contiguous)
            cb = 0
            for s in range(NBLK):
                blo = base_node + s * P
                bhi = min(blo + P, base_node + NPC)
                lo = np.searchsorted(dst, blo, side="left")
                hi = np.searchsorted(dst, bhi, side="left")
                sk = skey[lo:hi]
                dr = (dst[lo:hi] - blo).astype(np.float32)
                a_mask = sk < HALF
                for which, KK, pad in ((a_mask, Ka[s], 0),
                                       (~a_mask, Kb[s], HALF)):
                    cnt = int(which.sum())
                    pos = cb * P
                    SRCK[c, pos:pos + cnt] = sk[which]
                    # pad indices must stay valid for the table half
                    SRCK[c, pos + cnt:(cb + int(KK)) * P] = pad
                    DREL[c, pos:pos + cnt] = dr[which]
                    cb += int(KK)
        # device arrays
        DREL_t = np.ascontiguousarray(
            DREL.reshape(NCORES, NCH, P).transpose(0, 2, 1))
        DRELT = np.ascontiguousarray(DREL.reshape(NCORES, 1, EPAD))
        # wrapped int16 indices per gather op, [128, NOPS*64]
        IDXW = np.zeros((NCORES, P, NOPS * GRP * 8), dtype=np.int16)
        for c in range(NCORES):
            for o, (c0, ncg, t) in enumerate(ops):
                iv = SRCK[c, c0 * P:(c0 + ncg) * P] - (HALF if t else 0)
                w = iv.reshape(-1, 16).T.astype(np.int16)  # [16, n/16]
                IDXW[c, :, o * GRP * 8: o * GRP * 8 + w.shape[1]] = \
                    np.tile(w, (8, 1))
        return dict(NCH=NCH, meta=meta, ops=ops, NOPS=NOPS,
                    Ktot=[int(Ka[s] + Kb[s]) for s in range(NBLK)],
                    DREL=DREL_t, DRELT=DRELT, IDXW=IDXW)

    L1 = build_layer(1)
    L2 = build_layer(2)

    # per-slot block-node gather indices (alpha_dst blocks)
    BLKI = np.zeros((NCORES, P, NBLK), dtype=np.int32)
    BLKI2 = np.zeros((NCORES, P, NBLK), dtype=np.int32)
    for c in range(NCORES):
        for s in range(NBLK):
            nodes = np.minimum(c * NPC + s * P + np.arange(P), N - 1)
            BLKI[c, :, s] = nodes
            BLKI2[c, :, s] = (nodes // NPC) * NROWC + (nodes % NPC)

    xT = np.zeros((D, NROW1), dtype=np.float32)
    xT[:, :N] = x.T
    A1 = np.zeros((F1, 6), dtype=np.float32)
    for h in range(H):
        A1[h * HID:(h + 1) * HID, h] = as1[h]
        A1[h * HID:(h + 1) * HID, 3 + h] = ad1[h]
    A2 = np.stack([as2[0], ad2[0]], axis=1).astype(np.float32)

    shared = {
        "xT": xT,
        "W1": np.ascontiguousarray(W1),
        "W1T": np.ascontiguousarray(W1.T),
        "A1": A1,
        "W2": np.ascontiguousarray(W2),
        "W2T": np.ascontiguousarray(W2.T),
        "A2": A2,
        "B1": np.ascontiguousarray(np.broadcast_to(b1, (P, F1))),
        "B2": np.ascontiguousarray(np.broadcast_to(b2, (P, F2))),
        "IOTA": np.ascontiguousarray(
            np.broadcast_to(np.arange(P, dtype=np.float32), (P, P))),
        "IOTAC": np.arange(P, dtype=np.float32).reshape(P, 1),
    }
    percore = []
    for c in range(NCORES):
        percore.append({
            "DREL1": L1["DREL"][c], "DRELT1": L1["DRELT"][c],
            "IDXW1": L1["IDXW"][c],
            "DREL2": L2["DREL"][c], "DRELT2": L2["DRELT"][c],
            "IDXW2": L2["IDXW"][c],
            "BLKI": BLKI[c], "BLKI2": BLKI2[c],
        })
    key = (tuple(L1["Ktot"]), tuple(x[0] for x in L1["ops"]),
           tuple(x[1] for x in L1["ops"]), tuple(x[2] for x in L1["ops"]),
           tuple(L2["Ktot"]), tuple(x[0] for x in L2["ops"]),
           tuple(x[1] for x in L2["ops"]), tuple(x[2] for x in L2["ops"]))
    return key, (L1, L2), shared, percore


def _ap_view(ap, extra_offset, free_dims):
    import concourse.bass as bass

    return bass.AP(
        tensor=ap.tensor, offset=ap.offset + extra_offset,
        ap=[list(ap.ap[0])] + [list(d) for d in free_dims],
    )


def _build(L1, L2):
    import concourse.bass as bass
    import concourse.bacc as bacc
    import concourse.tile as tile
    from concourse import mybir
    from concourse.masks import make_identity
    from concourse.library_config import mlp
    from contextlib import ExitStack

    f32 = mybir.dt.float32
    bf16 = mybir.dt.bfloat16
    i32 = mybir.dt.int32
    i16 = mybir.dt.int16
    AT = mybir.ActivationFunctionType
    OP = mybir.AluOpType
    IOA = bass.IndirectOffsetOnAxis

    nc = bacc.Bacc("TRN2", target_bir_lowering=False, debug=False,
                   num_devices=NCORES, num_swdge_queues=4)

    xT = nc.dram_tensor("xT", [D, NROW1], f32, kind="ExternalInput")
    W1 = nc.dram_tensor("W1", [D, F1], f32, kind="ExternalInput")
    W1T = nc.dram_tensor("W1T", [F1, D], f32, kind="ExternalInput")
    A1 = nc.dram_tensor("A1", [F1, 6], f32, kind="ExternalInput")
    W2 = nc.dram_tensor("W2", [F1, F2], f32, kind="ExternalInput")
    W2T = nc.dram_tensor("W2T", [F2, F1], f32, kind="ExternalInput")
    A2 = nc.dram_tensor("A2", [F2, 2], f32, kind="ExternalInput")
    B1 = nc.dram_tensor("B1", [P, F1], f32, kind="ExternalInput")
    B2 = nc.dram_tensor("B2", [P, F2], f32, kind="ExternalInput")
    IOTA = nc.dram_tensor("IOTA", [P, P], f32, kind="ExternalInput")
    IOTAC = nc.dram_tensor("IOTAC", [P, 1], f32, kind="ExternalInput")
    DREL1 = nc.dram_tensor("DREL1", [P, L1["NCH"]], f32, kind="ExternalInput")
    DRELT1 = nc.dram_tensor("DRELT1", [1, L1["NCH"] * P], f32,
                            kind="ExternalInput")
    IDXW1 = nc.dram_tensor("IDXW1", [P, L1["NOPS"] * GRP * 8], i16,
                           kind="ExternalInput")
    DREL2 = nc.dram_tensor("DREL2", [P, L2["NCH"]], f32, kind="ExternalInput")
    DRELT2 = nc.dram_tensor("DRELT2", [1, L2["NCH"] * P], f32,
                            kind="ExternalInput")
    IDXW2 = nc.dram_tensor("IDXW2", [P, L2["NOPS"] * GRP * 8], i16,
                           kind="ExternalInput")
    BLKI = nc.dram_tensor("BLKI", [P, NBLK], i32, kind="ExternalInput")
    BLKI2 = nc.dram_tensor("BLKI2", [P, NBLK], i32, kind="ExternalInput")
    OUT = nc.dram_tensor("out", [NROWC, F2], f32, kind="ExternalOutput")

    G1a = nc.dram_tensor("G1a", [HALF, G1W], bf16, kind="Internal")
    G1b = nc.dram_tensor("G1b", [NROW1 - HALF, G1W], bf16, kind="Internal")
    AD1 = nc.dram_tensor("AD1", [NROW1, 4], f32, kind="Internal")
    G2L = nc.dram_tensor("G2L", [NROWC, G2W], bf16, kind="Internal")
    AD2L = nc.dram_tensor("AD2L", [NROWC, 2], f32, kind="Internal")
    G2F = nc.dram_tensor("G2F", [NROWC * NCORES, G2W], bf16,
                         addr_space="Shared", kind="Internal")
    AD2F = nc.dram_tensor("AD2F", [NROWC * NCORES, 2], f32,
                          addr_space="Shared", kind="Internal")

    with tile.TileContext(nc) as tc, ExitStack() as ctx:
        consts = ctx.enter_context(tc.tile_pool(name="consts", bufs=1))
        sbA = ctx.enter_context(tc.tile_pool(name="sbA", bufs=6))
        psum = ctx.enter_context(tc.tile_pool(name="psum", bufs=3, space="PSUM"))
        psbc = ctx.enter_context(tc.tile_pool(name="psbc", bufs=1, space="PSUM"))
        psad = ctx.enter_context(tc.tile_pool(name="psad", bufs=2, space="PSUM"))
        pst = ctx.enter_context(tc.tile_pool(name="pst", bufs=1, space="PSUM"))
        gpool = ctx.enter_context(tc.tile_pool(name="gpool", bufs=4))
        fpool = ctx.enter_context(tc.tile_pool(name="fpool", bufs=4))
        spool = ctx.enter_context(tc.tile_pool(name="spool", bufs=4))
        ipool = ctx.enter_context(tc.tile_pool(name="ipool", bufs=4))
        epool = ctx.enter_context(tc.tile_pool(name="epool", bufs=3))

        nc.gpsimd.load_library(mlp)

        # ---------------- constants / weight prep ----------------
        iota = consts.tile([P, P], f32)
        nc.sync.dma_start(out=iota[:], in_=IOTA[:])
        iotac = consts.tile([P, 1], f32)
        nc.sync.dma_start(out=iotac[:], in_=IOTAC[:])
        ones1 = consts.tile([1, P], f32)
        nc.vector.memset(ones1[:], 1.0)
        b1t = consts.tile([P, F1], f32)
        nc.sync.dma_start(out=b1t[:], in_=B1[:])
        b2t = consts.tile([P, F2], f32)
        nc.sync.dma_start(out=b2t[:], in_=B2[:])
        ident = consts.tile([P, P], f32)
        make_identity(nc, ident[:])

        rhs1 = consts.tile([P, 198], f32)
        nc.sync.dma_start(out=rhs1[:, :F1], in_=W1[:])
        w1t_a = consts.tile([P, D], f32)
        nc.sync.dma_start(out=w1t_a[:], in_=W1T[0:P, :])
        w1t_b = consts.tile([F1 - P, D], f32)
        nc.sync.dma_start(out=w1t_b[:], in_=W1T[P:F1, :])
        a1_a = consts.tile([P, 6], f32)
        nc.sync.dma_start(out=a1_a[:], in_=A1[0:P, :])
        a1_b = consts.tile([F1 - P, 6], f32)
        nc.sync.dma_start(out=a1_b[:], in_=A1[P:F1, :])
        pu = pst.tile([P, P], f32, tag="tr")
        nc.tensor.matmul(out=pu[:, :6], lhsT=w1t_a[:], rhs=a1_a[:],
                         start=True, stop=False)
        nc.tensor.matmul(out=pu[:, :6], lhsT=w1t_b[:], rhs=a1_b[:],
                         start=False, stop=True)
        nc.vector.tensor_copy(out=rhs1[:, F1:F1 + 6], in_=pu[:, :6])

        w2t = consts.tile([F2, F1], f32)
        nc.sync.dma_start(out=w2t[:], in_=W2T[:])
        a2t = consts.tile([F2, 2], f32)
        nc.sync.dma_start(out=a2t[:], in_=A2[:])
        rhs2_lo = consts.tile([P, 66], f32)
        nc.sync.dma_start(out=rhs2_lo[:, :F2], in_=W2[0:P, :])
        rhs2_hi = consts.tile([F1 - P, 66], f32)
        nc.sync.dma_start(out=rhs2_hi[:, :F2], in_=W2[P:F1, :])
        pu2a = pst.tile([P, P], f32, tag="tr")
        nc.tensor.matmul(out=pu2a[:, :2], lhsT=w2t[:, 0:P], rhs=a2t[:],
                         start=True, stop=True)
        nc.vector.tensor_copy(out=rhs2_lo[:, F2:F2 + 2], in_=pu2a[:, :2])
        pu2b = pst.tile([F1 - P, P], f32, tag="tr2")
        nc.tensor.matmul(out=pu2b[:, :2], lhsT=w2t[:, P:F1], rhs=a2t[:],
                         start=True, stop=True)
        nc.vector.tensor_copy(out=rhs2_hi[:, F2:F2 + 2], in_=pu2b[:, :2])

        # ---------------- stage A ----------------
        for t in range(NT):
            xt = sbA.tile([P, P], f32, tag="xt")
            nc.sync.dma_start(out=xt[:], in_=xT[:, t * P:(t + 1) * P])
            pa = psum.tile([P, 200], f32, tag="mm")
            nc.tensor.matmul(out=pa[:, :198], lhsT=xt[:], rhs=rhs1[:],
                             start=True, stop=True)
            gbf = sbA.tile([P, G1W], bf16, tag="gbf")
            nc.vector.tensor_copy(out=gbf[:, :F1], in_=pa[:, :F1])
            gf32 = gbf[:].bitcast(f32)
            nc.vector.tensor_copy(out=gf32[:, 96:99], in_=pa[:, F1:F1 + 3])
            adw = sbA.tile([P, 4], f32, tag="adw")
            nc.vector.tensor_copy(out=adw[:, :3], in_=pa[:, F1 + 3:F1 + 6])
            if t < HALF // P:
                nc.scalar.dma_start(out=G1a[t * P:(t + 1) * P, :], in_=gbf[:])
            else:
                tb = t - HALF // P
                nc.scalar.dma_start(out=G1b[tb * P:(tb + 1) * P, :], in_=gbf[:])
            nc.scalar.dma_start(out=AD1[t * P:(t + 1) * P, :3], in_=adw[:, :3])

        # ---------------- generic edge phase ----------------
        def edge_layer(LM, TBLa, TBLb, width, nfeat, as_f32col, ADT, adw_,
                       adcol, dreli, drelti, idxwi, blki, ps_width,
                       slot_epilogue):
            nheads = ps_width - nfeat
            NCH = LM["NCH"]
            meta = LM["meta"]
            ops = LM["ops"]
            Ktot = LM["Ktot"]
            blkit = consts.tile([P, NBLK], i32, name=f"blkit{nfeat}")
            nc.sync.dma_start(out=blkit[:], in_=blki[:])
            ps_cur = [None]
            adb_cur = [None]
            fw = nfeat + nheads  # F8 row width

            def new_slot(s):
                adb = epool.tile([P, 4], f32, tag="adb", name="adb")
                nc.gpsimd.indirect_dma_start(
                    out=adb[:, :adw_], out_offset=None, in_=ADT[:],
                    in_offset=IOA(ap=blkit[:, s:s + 1], axis=0))
                adbh = epool.tile([P, 4], bf16, tag="adbh", name="adbh")
                nc.vector.tensor_copy(out=adbh[:, :adw_], in_=adb[:, :adw_])
                adb_cur[0] = adbh
                ps_cur[0] = psum.tile([P, 200], f32, tag="mm", name="ps_slot")

            for o, (c0, ncg, tb) in enumerate(ops):
                idxt = ipool.tile([P, GRP * 8], i16, tag="idxt", name="idxt")
                nc.sync.dma_start(
                    out=idxt[:, :ncg * 8],
                    in_=idxwi[:, o * GRP * 8:o * GRP * 8 + ncg * 8])
                drt = ipool.tile([P, GRP], f32, tag="drt", name="drt")
                nc.sync.dma_start(out=drt[:, :ncg],
                                  in_=dreli[:, c0:c0 + ncg])
                grow = gpool.tile([P, GRP, width], bf16, tag="grow",
                                  name="grow")
                nidx = ncg * P
                nc.gpsimd.dma_gather(
                    grow[:, :ncg, :], (TBLb if tb else TBLa)[:],
                    idxt[:, :ncg * 8], nidx, nidx, width,
                    queue_num=o % 4)
                # S: [e_part, chunk, d] one-hot
                S8 = spool.tile([P, GRP * P], bf16, tag="s8", name="s8")
                nc.vector.tensor_tensor(
                    out=_ap_view(S8[:], 0, [[P, ncg], [1, P]]),
                    in0=_ap_view(drt[:], 0, [[1, ncg], [0, P]]),
                    in1=_ap_view(iota[:], 0, [[0, ncg], [1, P]]),
                    op=OP.is_equal)
                # S_T + alpha_dst expansion (per SUB-chunk batches)
                adp = psad.tile([P, GRP * nheads], f32, tag="adp", name="adp")
                j = 0
                while j < ncg:
                    sb = min(SUB, ncg - j)
                    drl = ipool.tile([1, SUB * P], f32, tag="drl", name="drl")
                    nc.scalar.dma_start(
                        out=drl[:, :sb * P],
                        in_=drelti[:, (c0 + j) * P:(c0 + j + sb) * P])
                    pbc = psbc.tile([P, SUB * P], f32, tag="bc", name="pbc")
                    nc.tensor.matmul(out=pbc[:, :sb * P], lhsT=ones1[:],
                                     rhs=drl[:, :sb * P], start=True, stop=True)
                    st8 = spool.tile([P, SUB * P], bf16, tag="st8", name="st8")
                    nc.vector.tensor_scalar(
                        out=st8[:, :sb * P], in0=pbc[:, :sb * P],
                        scalar1=iotac[:, :1], scalar2=None, op0=OP.is_equal)
                    for jj in range(sb):
                        s, k, _tb2 = meta[c0 + j + jj]
                        if k == 0:
                            new_slot(s)
                        nc.tensor.matmul(
                            out=adp[:, (j + jj) * nheads:(j + jj + 1) * nheads],
                            lhsT=st8[:, jj * P:(jj + 1) * P],
                            rhs=adb_cur[0][:, adcol:adcol + nheads],
                            start=True, stop=True)
                    j += sb
                # logits -> exp -> weighted features
                growf = grow[:].bitcast(f32)
                t8 = epool.tile([P, GRP * nheads], f32, tag="t8", name="t8")
                nc.vector.tensor_tensor(
                    out=_ap_view(t8[:], 0, [[nheads, ncg], [1, nheads]]),
                    in0=_ap_view(growf, as_f32col,
                                 [[width // 2, ncg], [1, nheads]]),
                    in1=_ap_view(adp[:], 0, [[nheads, ncg], [1, nheads]]),
                    op=OP.add)
                r8 = epool.tile([P, GRP * nheads], f32, tag="r8", name="r8")
                nc.vector.tensor_scalar(
                    out=r8[:, :ncg * nheads], in0=t8[:, :ncg * nheads],
                    scalar1=0.0, scalar2=SLOPE, op0=OP.min, op1=OP.mult)
                l8 = epool.tile([P, GRP * nheads], f32, tag="l8", name="l8")
                nc.vector.scalar_tensor_tensor(
                    out=l8[:, :ncg * nheads], in0=t8[:, :ncg * nheads],
                    scalar=0.0, in1=r8[:, :ncg * nheads],
                    op0=OP.max, op1=OP.add)
                F8 = fpool.tile([P, GRP * fw], bf16, tag="f8", name="f8")
                nc.scalar.activation(
                    out=_ap_view(F8[:], nfeat, [[fw, ncg], [1, nheads]]),
                    in_=_ap_view(l8[:], 0, [[nheads, ncg], [1, nheads]]),
                    func=AT.Exp)
                hd = nfeat // nheads
                nc.vector.tensor_tensor(
                    out=_ap_view(F8[:], 0, [[fw, ncg], [hd, nheads], [1, hd]]),
                    in0=_ap_view(grow[:], 0,
                                 [[width, ncg], [hd, nheads], [1, hd]]),
                    in1=_ap_view(F8[:], nfeat,
                                 [[fw, ncg], [1, nheads], [0, hd]]),
                    op=OP.mult)
                for jj in range(ncg):
                    s, k, _tb2 = meta[c0 + jj]
                    nc.tensor.matmul(
                        out=ps_cur[0][:, :ps_width],
                        lhsT=S8[:, jj * P:(jj + 1) * P],
                        rhs=F8[:, jj * fw:jj * fw + ps_width],
                        start=(k == 0), stop=(k == Ktot[s] - 1))
                    if k == Ktot[s] - 1:
                        slot_epilogue(s, ps_cur[0])

        # L1 epilogue: h -> transpose -> G2 rows + AD2
        def epi1(s, ps):
            rc = epool.tile([P, H], f32, tag="rc", name="rc")
            nc.vector.tensor_scalar_add(out=rc[:], in0=ps[:, F1:F1 + H],
                                        scalar1=EPS)
            rc2 = epool.tile([P, H], f32, tag="rc2", name="rc2")
            nc.vector.reciprocal(out=rc2[:], in_=rc[:])
            hm = epool.tile([P, F1], f32, tag="hm", name="hm")
            nc.vector.tensor_tensor(
                out=_ap_view(hm[:], 0, [[HID, H], [1, HID]]),
                in0=_ap_view(ps[:, :F1], 0, [[HID, H], [1, HID]]),
                in1=_ap_view(rc2[:], 0, [[1, H], [0, HID]]),
                op=OP.mult)
            hb = epool.tile([P, F1], f32, tag="hb", name="hb")
            nc.vector.tensor_tensor(out=hb[:], in0=hm[:], in1=b1t[:], op=OP.add)
            hr = epool.tile([P, F1], f32, tag="hr", name="hr")
            nc.scalar.activation(out=hr[:], in_=hb[:], func=AT.Relu)
            pt1 = pst.tile([P, P], f32, tag="tr", name="pt1")
            nc.tensor.transpose(out=pt1[:], in_=hr[:, :P], identity=ident[:])
            pt2 = pst.tile([F1 - P, P], f32, tag="tr2", name="pt2")
            nc.tensor.transpose(out=pt2[:], in_=hr[:, P:F1], identity=ident[:])
            ht1 = epool.tile([P, P], f32, tag="ht1", name="ht1")
            nc.vector.tensor_copy(out=ht1[:], in_=pt1[:])
            ht2 = epool.tile([F1 - P, P], f32, tag="ht2", name="ht2")
            nc.vector.tensor_copy(out=ht2[:], in_=pt2[:])
            pg = psum.tile([P, 200], f32, tag="mm", name="pg")
            nc.tensor.matmul(out=pg[:, :66], lhsT=ht1[:], rhs=rhs2_lo[:],
                             start=True, stop=False)
            nc.tensor.matmul(out=pg[:, :66], lhsT=ht2[:], rhs=rhs2_hi[:],
                             start=False, stop=True)
            g2 = epool.tile([P, G2W], bf16, tag="g2", name="g2")
            nc.vector.tensor_copy(out=g2[:, :F2], in_=pg[:, :F2])
            g2f = g2[:].bitcast(f32)
            nc.vector.tensor_copy(out=g2f[:, 32:33], in_=pg[:, F2:F2 + 1])
            ad2w = epool.tile([P, 2], f32, tag="ad2w", name="ad2w")
            nc.vector.tensor_copy(out=ad2w[:, :1], in_=pg[:, F2 + 1:F2 + 2])
            nc.sync.dma_start(out=G2L[s * P:(s + 1) * P, :], in_=g2[:])
            nc.sync.dma_start(out=AD2L[s * P:(s + 1) * P, :1], in_=ad2w[:, :1])

        edge_layer(L1, G1a, G1b, G1W, F1, 96, AD1, 3, 0,
                   DREL1, DRELT1, IDXW1, BLKI, F1 + H, epi1)

        # ---------------- AllGather ----------------
        nc.gpsimd.collective_compute(
            "AllGather", mybir.AluOpType.bypass,
            replica_groups=[list(range(NCORES))],
            ins=[G2L.ap().opt()], outs=[G2F.ap().opt()])
        nc.gpsimd.collective_compute(
            "AllGather", mybir.AluOpType.bypass,
            replica_groups=[list(range(NCORES))],
            ins=[AD2L.ap().opt()], outs=[AD2F.ap().opt()])

        # ---------------- layer 2 ----------------
        def epi2(s, ps):
            rc = epool.tile([P, 1], f32, tag="rcB", name="rcB")
            nc.vector.tensor_scalar_add(out=rc[:], in0=ps[:, F2:F2 + 1],
                                        scalar1=EPS)
            rc2 = epool.tile([P, 1], f32, tag="rcB2", name="rcB2")
            nc.vector.reciprocal(out=rc2[:], in_=rc[:])
            om = epool.tile([P, F2], f32, tag="om", name="om")
            nc.vector.tensor_tensor(out=om[:], in0=ps[:, :F2],
                                    in1=rc2[:].to_broadcast([P, F2]),
                                    op=OP.mult)
            ob = epool.tile([P, F2], f32, tag="ob", name="ob")
            nc.vector.tensor_tensor(out=ob[:], in0=om[:], in1=b2t[:], op=OP.add)
            orl = epool.tile([P, F2], f32, tag="orl", name="orl")
            nc.scalar.activation(out=orl[:], in_=ob[:], func=AT.Relu)
            nc.sync.dma_start(out=OUT[s * P:(s + 1) * P, :], in_=orl[:])

        # G2F views for the two index halves (offsets stay < 2^24 bytes)
        g2fa = G2F[0:HALF, :]
        g2fb = G2F[HALF:NROWC * NCORES, :]
        edge_layer(L2, g2fa, g2fb, G2W, F2, 32, AD2F, 1, 0,
                   DREL2, DRELT2, IDXW2, BLKI2, F2 + 1, epi2)

    nc.compile()
    return nc


def _get_compiled(key, layers):
    if key not in _compiled:
        _compiled[key] = _build(layers[0], layers[1])
    return _compiled[key]


def run(inputs, **runkw):
    from concourse import bass_utils

    key, layers, shared, percore = _host_prep(inputs)
    nc = _get_compiled(key, layers)
    in_maps = []
    for c in range(NCORES):
        m = dict(shared)
        m.update(percore[c])
        in_maps.append(m)
    res = bass_utils.run_bass_kernel_spmd(
        nc, in_maps, core_ids=list(range(NCORES)), **runkw)
    return res


def assemble(results):
    out = np.empty((N, F2), dtype=np.float32)
    for c in range(NCORES):
        out[c * NPC:(c + 1) * NPC] = results[c]["out"][:NPC]
    return out


def kernel(**inputs):
    res = run(inputs)
    return assemble(res.results)


# revision 47
# speedup vs baseline: 1.7664x; 1.0222x over previous
"""GAT 2-layer kernel for Trainium2, 8 NeuronCores (SPMD, dst-sharded).

Strategy (v3):
  - Destination-node sharding: core c owns nodes [c*6250,(c+1)*6250); edges bucketed
    into per-128-dst-node "slots", padded to 128-edge chunks.
  - Stage A (replicated): per 128-node tile one matmul computes
    [x@W1 (192) | alpha_src (3) | alpha_dst (3)]; xw+as go to a bf16 gather table
    G1 (512B rows: 192 bf16 xw + 3 f32 alpha_src bit-packed + pad), ad to slim f32
    table AD1. G1 is split into two <=32768-row tensors (dma_gather int16 index
    limit, 16MiB ucode offset limit).
  - Edge phase per layer: per <=8-chunk group one dma_gather (1024 row gathers/op)
    pulls source rows; one-hot S (DVE is_equal vs iota) segment-reduces
    exp(logit)-weighted features AND the softmax denominators in one per-slot PSUM
    accumulation (normalization pulled out of the sum; logits are O(5), no
    max-subtraction needed). alpha_dst[dst] is expanded edge-wise on the PE:
    one-hot-transpose S_T (built by K=1 ones-matmul broadcast of dst_rel + DVE
    compare) times the slot's alpha_dst block (gathered once per slot).
  - Per-slot epilogue: h = relu(sum/(denom+eps) + bias1); PE-transpose h and
    immediately emit G2 rows [h@W2 (64) bf16 | as2 f32] and slim AD2; AllGather
    both; layer 2 repeats the edge phase (1 head) against G2F views.
"""
import sys

sys.path.insert(0, "/opt/trn_rl_repo")
import numpy as np
import ml_dtypes

N = 50000
D = 128
HID = 64
H = 3
F1 = 192
F2 = 64
NCORES = 8
NPC = N // NCORES          # 6250 nodes per core
P = 128
NBLK = (NPC + P - 1) // P  # 49 slots per core
NT = (N + P - 1) // P      # 391 stage-A node tiles
NROW1 = NT * P             # 50048 G1 rows
HALF = 32768               # dma_gather int16 index limit
G1W = 256                  # bf16 cols: xw(192) | as f32 x3 (bf16 192:198) | pad
G2W = 128                  # bf16 cols: xw2(64) | as2 f32 (bf16 64:66) | pad
NROWC = NBLK * P           # 6272 rows per core shard
SLOPE = 0.2
EPS = 1e-16
GRP = 8                    # max chunks per dma_gather / op group
SUB = 4                    # chunks per S_T broadcast matmul (512 psum cols)

_compiled = {}


def _chunkize(src_key, dst, order_all):
    """Bucket edges per (core, slot), split by src_key half, pad to 128.
    Returns per-core flat arrays + compile-time chunk structure (shared)."""
    core = dst // NPC
    rel = dst % NPC
    slot = rel // P
    half = (src_key >= HALF).astype(np.int64)
    # counts[core, slot, half]
    counts = np.zeros((NCORES, NBLK, 2), dtype=np.int64)
    np.add.at(counts, (core, slot, half), 1)
    Ka = np.ceil(counts[:, :, 0] / P).astype(np.int64).max(axis=0)
    Kb = np.ceil(counts[:, :, 1] / P).astype(np.int64).max(axis=0)
    return Ka, Kb


def _host_prep(inputs):
    x = np.asarray(inputs["x"], dtype=np.float32)
    ei = np.asarray(inputs["edge_index"])
    W1 = np.asarray(inputs["W1"], dtype=np.float32)
    as1 = np.asarray(inputs["att_src1"], dtype=np.float32)
    ad1 = np.asarray(inputs["att_dst1"], dtype=np.float32)
    b1 = np.asarray(inputs["bias1"], dtype=np.float32)
    W2 = np.asarray(inputs["W2"], dtype=np.float32)
    as2 = np.asarray(inputs["att_src2"], dtype=np.float32)
    ad2 = np.asarray(inputs["att_dst2"], dtype=np.float32)
    b2 = np.asarray(inputs["bias2"], dtype=np.float32)

    loops = np.arange(N, dtype=np.int64)
    src = np.concatenate([ei[0].astype(np.int64), loops])
    dst = np.concatenate([ei[1].astype(np.int64), loops])
    order = np.argsort(dst, kind="stable")
    src = src[order]
    dst = dst[order]
    g2row = (src // NPC) * NROWC + (src % NPC)

    # chunk structure per layer (uniform across cores)
    Ka1, Kb1 = _chunkize(src, dst, None)
    Ka2, Kb2 = _chunkize(g2row, dst, None)

    def build_layer(key):
        Ka, Kb = (Ka1, Kb1) if key == 1 else (Ka2, Kb2)
        skey = src if key == 1 else g2row
        NCH = int((Ka + Kb).sum())
        # chunk meta: (slot, k_in_slot, table) in processing order
        # b-table chunks first: their gathers only depend on the (smaller,
        # first-written) G1b table, overlapping the tail of stage A
        meta = []
        for s in range(NBLK):
            k = 0
            for _ in range(int(Kb[s])):
                meta.append((s, k, 1)); k += 1
            for _ in range(int(Ka[s])):
                meta.append((s, k, 0)); k += 1
        # gather ops: runs of <=GRP same-table consecutive chunks
        ops = []   # (chunk_start, n_chunks, table)
        i = 0
        while i < NCH:
            t = meta[i][2]
            j = i
            while j < NCH and j - i < GRP and meta[j][2] == t:
                j += 1
            ops.append((i, j - i, t))
            i = j
        NOPS = len(ops)

        EPAD = NCH * P
        SRCK = np.zeros((NCORES, EPAD), dtype=np.int64)
        DREL = np.full((NCORES, EPAD), 255.0, dtype=np.float32)
        for c in range(NCORES):
            base_node = c * NPC
            # per-slot edge ranges (dst-sorted => contiguous)
            cb = 0
            for s in range(NBLK):
                blo = base_node + s * P
                bhi = min(blo + P, base_node + NPC)
                lo = np.searchsorted(dst, blo, side="left")
                hi = np.searchsorted(dst, bhi, side="left")
                sk = skey[lo:hi]
                dr = (dst[lo:hi] - blo).astype(np.float32)
                a_mask = sk < HALF
                for which, KK, pad in ((~a_mask, Kb[s], HALF),
                                       (a_mask, Ka[s], 0)):
                    cnt = int(which.sum())
                    pos = cb * P
                    SRCK[c, pos:pos + cnt] = sk[which]
                    # pad indices must stay valid for the table half
                    SRCK[c, pos + cnt:(cb + int(KK)) * P] = pad
                    DREL[c, pos:pos + cnt] = dr[which]
                    cb += int(KK)
        # device arrays
        DREL_t = np.ascontiguousarray(
            DREL.reshape(NCORES, NCH, P).transpose(0, 2, 1))
        DRELT = np.ascontiguousarray(DREL.reshape(NCORES, 1, EPAD))
        # wrapped int16 indices per gather op, [128, NOPS*64]
        IDXW = np.zeros((NCORES, P, NOPS * GRP * 8), dtype=np.int16)
        for c in range(NCORES):
            for o, (c0, ncg, t) in enumerate(ops):
                iv = SRCK[c, c0 * P:(c0 + ncg) * P] - (HALF if t else 0)
                w = iv.reshape(-1, 16).T.astype(np.int16)  # [16, n/16]
                IDXW[c, :, o * GRP * 8: o * GRP * 8 + w.shape[1]] = \
                    np.tile(w, (8, 1))
        return dict(NCH=NCH, meta=meta, ops=ops, NOPS=NOPS,
                    Ktot=[int(Ka[s] + Kb[s]) for s in range(NBLK)],
                    DREL=DREL_t, DRELT=DRELT, IDXW=IDXW)

    L1 = build_layer(1)
    L2 = build_layer(2)

    # per-slot block-node gather indices (alpha_dst blocks)
    BLKI = np.zeros((NCORES, P, NBLK), dtype=np.int32)
    BLKI2 = np.zeros((NCORES, P, NBLK), dtype=np.int32)
    for c in range(NCORES):
        for s in range(NBLK):
            nodes = np.minimum(c * NPC + s * P + np.arange(P), N - 1)
            BLKI[c, :, s] = nodes
            BLKI2[c, :, s] = (nodes // NPC) * NROWC + (nodes % NPC)

    xT = np.zeros((D, NROW1), dtype=np.float32)
    xT[:, :N] = x.T
    A1 = np.zeros((F1, 6), dtype=np.float32)
    for h in range(H):
        A1[h * HID:(h + 1) * HID, h] = as1[h]
        A1[h * HID:(h + 1) * HID, 3 + h] = ad1[h]
    A2 = np.stack([as2[0], ad2[0]], axis=1).astype(np.float32)

    shared = {
        "xT": xT,
        "W1": np.ascontiguousarray(W1),
        "W1T": np.ascontiguousarray(W1.T),
        "A1": A1,
        "W2": np.ascontiguousarray(W2),
        "W2T": np.ascontiguousarray(W2.T),
        "A2": A2,
        "B1": np.ascontiguousarray(np.broadcast_to(b1, (P, F1))),
        "B2": np.ascontiguousarray(np.broadcast_to(b2, (P, F2))),
        "IOTA": np.ascontiguousarray(
            np.broadcast_to(np.arange(P, dtype=np.float32), (P, P))),
        "IOTAC": np.arange(P, dtype=np.float32).reshape(P, 1),
    }
    percore = []
    for c in range(NCORES):
        percore.append({
            "DREL1": L1["DREL"][c], "DRELT1": L1["DRELT"][c],
            "IDXW1": L1["IDXW"][c],
            "DREL2": L2["DREL"][c], "DRELT2": L2["DRELT"][c],
            "IDXW2": L2["IDXW"][c],
            "BLKI": BLKI[c], "BLKI2": BLKI2[c],
        })
    key = (tuple(L1["Ktot"]), tuple(x[0] for x in L1["ops"]),
           tuple(x[1] for x in L1["ops"]), tuple(x[2] for x in L1["ops"]),
           tuple(L2["Ktot"]), tuple(x[0] for x in L2["ops"]),
           tuple(x[1] for x in L2["ops"]), tuple(x[2] for x in L2["ops"]))
    return key, (L1, L2), shared, percore


def _ap_view(ap, extra_offset, free_dims):
    import concourse.bass as bass

    return bass.AP(
        tensor=ap.tensor, offset=ap.offset + extra_offset,
        ap=[list(ap.ap[0])] + [list(d) for d in free_dims],
    )


def _build(L1, L2):
    import concourse.bass as bass
    import concourse.bacc as bacc
    import concourse.tile as tile
    from concourse import mybir
    from concourse.masks import make_identity
    from concourse.library_config import mlp
    from contextlib import ExitStack

    f32 = mybir.dt.float32
    bf16 = mybir.dt.bfloat16
    i32 = mybir.dt.int32
    i16 = mybir.dt.int16
    AT = mybir.ActivationFunctionType
    OP = mybir.AluOpType
    IOA = bass.IndirectOffsetOnAxis

    nc = bacc.Bacc("TRN2", target_bir_lowering=False, debug=False,
                   num_devices=NCORES, num_swdge_queues=4)

    xT = nc.dram_tensor("xT", [D, NROW1], f32, kind="ExternalInput")
    W1 = nc.dram_tensor("W1", [D, F1], f32, kind="ExternalInput")
    W1T = nc.dram_tensor("W1T", [F1, D], f32, kind="ExternalInput")
    A1 = nc.dram_tensor("A1", [F1, 6], f32, kind="ExternalInput")
    W2 = nc.dram_tensor("W2", [F1, F2], f32, kind="ExternalInput")
    W2T = nc.dram_tensor("W2T", [F2, F1], f32, kind="ExternalInput")
    A2 = nc.dram_tensor("A2", [F2, 2], f32, kind="ExternalInput")
    B1 = nc.dram_tensor("B1", [P, F1], f32, kind="ExternalInput")
    B2 = nc.dram_tensor("B2", [P, F2], f32, kind="ExternalInput")
    IOTA = nc.dram_tensor("IOTA", [P, P], f32, kind="ExternalInput")
    IOTAC = nc.dram_tensor("IOTAC", [P, 1], f32, kind="ExternalInput")
    DREL1 = nc.dram_tensor("DREL1", [P, L1["NCH"]], f32, kind="ExternalInput")
    DRELT1 = nc.dram_tensor("DRELT1", [1, L1["NCH"] * P], f32,
                            kind="ExternalInput")
    IDXW1 = nc.dram_tensor("IDXW1", [P, L1["NOPS"] * GRP * 8], i16,
                           kind="ExternalInput")
    DREL2 = nc.dram_tensor("DREL2", [P, L2["NCH"]], f32, kind="ExternalInput")
    DRELT2 = nc.dram_tensor("DRELT2", [1, L2["NCH"] * P], f32,
                            kind="ExternalInput")
    IDXW2 = nc.dram_tensor("IDXW2", [P, L2["NOPS"] * GRP * 8], i16,
                           kind="ExternalInput")
    BLKI = nc.dram_tensor("BLKI", [P, NBLK], i32, kind="ExternalInput")
    BLKI2 = nc.dram_tensor("BLKI2", [P, NBLK], i32, kind="ExternalInput")
    OUT = nc.dram_tensor("out", [NROWC, F2], f32, kind="ExternalOutput")

    G1a = nc.dram_tensor("G1a", [HALF, G1W], bf16, kind="Internal")
    G1b = nc.dram_tensor("G1b", [NROW1 - HALF, G1W], bf16, kind="Internal")
    AD1 = nc.dram_tensor("AD1", [NROW1, 4], f32, kind="Internal")
    G2L = nc.dram_tensor("G2L", [NROWC, G2W], bf16, kind="Internal")
    AD2L = nc.dram_tensor("AD2L", [NROWC, 2], f32, kind="Internal")
    G2F = nc.dram_tensor("G2F", [NROWC * NCORES, G2W], bf16,
                         addr_space="Shared", kind="Internal")
    AD2F = nc.dram_tensor("AD2F", [NROWC * NCORES, 2], f32,
                          addr_space="Shared", kind="Internal")

    with tile.TileContext(nc) as tc, ExitStack() as ctx:
        consts = ctx.enter_context(tc.tile_pool(name="consts", bufs=1))
        sbA = ctx.enter_context(tc.tile_pool(name="sbA", bufs=6))
        psum = ctx.enter_context(tc.tile_pool(name="psum", bufs=3, space="PSUM"))
        psbc = ctx.enter_context(tc.tile_pool(name="psbc", bufs=1, space="PSUM"))
        psad = ctx.enter_context(tc.tile_pool(name="psad", bufs=2, space="PSUM"))
        pst = ctx.enter_context(tc.tile_pool(name="pst", bufs=1, space="PSUM"))
        gpool = ctx.enter_context(tc.tile_pool(name="gpool", bufs=4))
        fpool = ctx.enter_context(tc.tile_pool(name="fpool", bufs=4))
        spool = ctx.enter_context(tc.tile_pool(name="spool", bufs=4))
        ipool = ctx.enter_context(tc.tile_pool(name="ipool", bufs=4))
        epool = ctx.enter_context(tc.tile_pool(name="epool", bufs=3))

        nc.gpsimd.load_library(mlp)

        # ---------------- constants / weight prep ----------------
        iota = consts.tile([P, P], f32)
        nc.sync.dma_start(out=iota[:], in_=IOTA[:])
        iotac = consts.tile([P, 1], f32)
        nc.sync.dma_start(out=iotac[:], in_=IOTAC[:])
        ones1 = consts.tile([1, P], f32)
        nc.vector.memset(ones1[:], 1.0)
        b1t = consts.tile([P, F1], f32)
        nc.sync.dma_start(out=b1t[:], in_=B1[:])
        b2t = consts.tile([P, F2], f32)
        nc.sync.dma_start(out=b2t[:], in_=B2[:])
        ident = consts.tile([P, P], f32)
        make_identity(nc, ident[:])

        rhs1 = consts.tile([P, 198], f32)
        nc.sync.dma_start(out=rhs1[:, :F1], in_=W1[:])
        w1t_a = consts.tile([P, D], f32)
        nc.sync.dma_start(out=w1t_a[:], in_=W1T[0:P, :])
        w1t_b = consts.tile([F1 - P, D], f32)
        nc.sync.dma_start(out=w1t_b[:], in_=W1T[P:F1, :])
        a1_a = consts.tile([P, 6], f32)
        nc.sync.dma_start(out=a1_a[:], in_=A1[0:P, :])
        a1_b = consts.tile([F1 - P, 6], f32)
        nc.sync.dma_start(out=a1_b[:], in_=A1[P:F1, :])
        pu = pst.tile([P, P], f32, tag="tr")
        nc.tensor.matmul(out=pu[:, :6], lhsT=w1t_a[:], rhs=a1_a[:],
                         start=True, stop=False)
        nc.tensor.matmul(out=pu[:, :6], lhsT=w1t_b[:], rhs=a1_b[:],
                         start=False, stop=True)
        nc.vector.tensor_copy(out=rhs1[:, F1:F1 + 6], in_=pu[:, :6])

        w2t = consts.tile([F2, F1], f32)
        nc.sync.dma_start(out=w2t[:], in_=W2T[:])
        a2t = consts.tile([F2, 2], f32)
        nc.sync.dma_start(out=a2t[:], in_=A2[:])
        rhs2_lo = consts.tile([P, 66], f32)
        nc.sync.dma_start(out=rhs2_lo[:, :F2], in_=W2[0:P, :])
        rhs2_hi = consts.tile([F1 - P, 66], f32)
        nc.sync.dma_start(out=rhs2_hi[:, :F2], in_=W2[P:F1, :])
        pu2a = pst.tile([P, P], f32, tag="tr")
        nc.tensor.matmul(out=pu2a[:, :2], lhsT=w2t[:, 0:P], rhs=a2t[:],
                         start=True, stop=True)
        nc.vector.tensor_copy(out=rhs2_lo[:, F2:F2 + 2], in_=pu2a[:, :2])
        pu2b = pst.tile([F1 - P, P], f32, tag="tr2")
        nc.tensor.matmul(out=pu2b[:, :2], lhsT=w2t[:, P:F1], rhs=a2t[:],
                         start=True, stop=True)
        nc.vector.tensor_copy(out=rhs2_hi[:, F2:F2 + 2], in_=pu2b[:, :2])

        # ---------------- stage A (G1b tiles first) ----------------
        for t in list(range(HALF // P, NT)) + list(range(HALF // P)):
            xt = sbA.tile([P, P], f32, tag="xt")
            nc.sync.dma_start(out=xt[:], in_=xT[:, t * P:(t + 1) * P])
            pa = psum.tile([P, 200], f32, tag="mm")
            nc.tensor.matmul(out=pa[:, :198], lhsT=xt[:], rhs=rhs1[:],
                             start=True, stop=True)
            gbf = sbA.tile([P, G1W], bf16, tag="gbf")
            nc.scalar.activation(out=gbf[:, :F1], in_=pa[:, :F1], func=AT.Copy)
            gf32 = gbf[:].bitcast(f32)
            nc.vector.tensor_copy(out=gf32[:, 96:99], in_=pa[:, F1:F1 + 3])
            adw = sbA.tile([P, 4], f32, tag="adw")
            nc.vector.tensor_copy(out=adw[:, :3], in_=pa[:, F1 + 3:F1 + 6])
            if t < HALF // P:
                nc.scalar.dma_start(out=G1a[t * P:(t + 1) * P, :], in_=gbf[:])
            else:
                tb = t - HALF // P
                nc.scalar.dma_start(out=G1b[tb * P:(tb + 1) * P, :], in_=gbf[:])
            nc.scalar.dma_start(out=AD1[t * P:(t + 1) * P, :3], in_=adw[:, :3])

        # ---------------- generic edge phase ----------------
        def edge_layer(LM, TBLa, TBLb, width, nfeat, as_f32col, ADT, adw_,
                       adcol, dreli, drelti, idxwi, blki, ps_width,
                       slot_epilogue):
            nheads = ps_width - nfeat
            NCH = LM["NCH"]
            meta = LM["meta"]
            ops = LM["ops"]
            Ktot = LM["Ktot"]
            blkit = consts.tile([P, NBLK], i32, name=f"blkit{nfeat}")
            nc.sync.dma_start(out=blkit[:], in_=blki[:])
            ps_cur = [None]
            adb_cur = [None]
            fw = nfeat + nheads  # F8 row width

            def new_slot(s):
                adb = epool.tile([P, 4], f32, tag="adb", name="adb")
                nc.gpsimd.indirect_dma_start(
                    out=adb[:, :adw_], out_offset=None, in_=ADT[:],
                    in_offset=IOA(ap=blkit[:, s:s + 1], axis=0))
                adbh = epool.tile([P, 4], bf16, tag="adbh", name="adbh")
                nc.vector.tensor_copy(out=adbh[:, :adw_], in_=adb[:, :adw_])
                adb_cur[0] = adbh
                ps_cur[0] = psum.tile([P, 200], f32, tag="mm", name="ps_slot")

            for o, (c0, ncg, tb) in enumerate(ops):
                idxt = ipool.tile([P, GRP * 8], i16, tag="idxt", name="idxt")
                nc.sync.dma_start(
                    out=idxt[:, :ncg * 8],
                    in_=idxwi[:, o * GRP * 8:o * GRP * 8 + ncg * 8])
                drt = ipool.tile([P, GRP], f32, tag="drt", name="drt")
                nc.sync.dma_start(out=drt[:, :ncg],
                                  in_=dreli[:, c0:c0 + ncg])
                grow = gpool.tile([P, GRP, width], bf16, tag="grow",
                                  name="grow")
                nidx = ncg * P
                nc.gpsimd.dma_gather(
                    grow[:, :ncg, :], (TBLb if tb else TBLa)[:],
                    idxt[:, :ncg * 8], nidx, nidx, width,
                    queue_num=o % 4)
                # S: [e_part, chunk, d] one-hot
                S8 = spool.tile([P, GRP * P], bf16, tag="s8", name="s8")
                nc.vector.tensor_tensor(
                    out=_ap_view(S8[:], 0, [[P, ncg], [1, P]]),
                    in0=_ap_view(drt[:], 0, [[1, ncg], [0, P]]),
                    in1=_ap_view(iota[:], 0, [[0, ncg], [1, P]]),
                    op=OP.is_equal)
                # S_T + alpha_dst expansion (per SUB-chunk batches)
                adp = psad.tile([P, GRP * nheads], f32, tag="adp", name="adp")
                j = 0
                while j < ncg:
                    sb = min(SUB, ncg - j)
                    drl = ipool.tile([1, SUB * P], f32, tag="drl", name="drl")
                    nc.scalar.dma_start(
                        out=drl[:, :sb * P],
                        in_=drelti[:, (c0 + j) * P:(c0 + j + sb) * P])
                    pbc = psbc.tile([P, SUB * P], f32, tag="bc", name="pbc")
                    nc.tensor.matmul(out=pbc[:, :sb * P], lhsT=ones1[:],
                                     rhs=drl[:, :sb * P], start=True, stop=True)
                    st8 = spool.tile([P, SUB * P], bf16, tag="st8", name="st8")
                    nc.vector.tensor_scalar(
                        out=st8[:, :sb * P], in0=pbc[:, :sb * P],
                        scalar1=iotac[:, :1], scalar2=None, op0=OP.is_equal)
                    for jj in range(sb):
                        s, k, _tb2 = meta[c0 + j + jj]
                        if k == 0:
                            new_slot(s)
                        nc.tensor.matmul(
                            out=adp[:, (j + jj) * nheads:(j + jj + 1) * nheads],
                            lhsT=st8[:, jj * P:(jj + 1) * P],
                            rhs=adb_cur[0][:, adcol:adcol + nheads],
                            start=True, stop=True)
                    j += sb
                # logits -> exp -> weighted features
                growf = grow[:].bitcast(f32)
                t8 = epool.tile([P, GRP * nheads], f32, tag="t8", name="t8")
                nc.vector.tensor_tensor(
                    out=_ap_view(t8[:], 0, [[nheads, ncg], [1, nheads]]),
                    in0=_ap_view(growf, as_f32col,
                                 [[width // 2, ncg], [1, nheads]]),
                    in1=_ap_view(adp[:], 0, [[nheads, ncg], [1, nheads]]),
                    op=OP.add)
                # exp(lrelu(t)) == max(exp(t), exp(SLOPE*t)) exactly
                e2 = epool.tile([P, GRP * nheads], f32, tag="r8", name="e2")
                nc.scalar.activation(out=e2[:, :ncg * nheads],
                                     in_=t8[:, :ncg * nheads],
                                     func=AT.Exp, scale=SLOPE)
                F8 = fpool.tile([P, GRP * fw], bf16, tag="f8", name="f8")
                nc.scalar.activation(
                    out=_ap_view(F8[:], nfeat, [[fw, ncg], [1, nheads]]),
                    in_=_ap_view(t8[:], 0, [[nheads, ncg], [1, nheads]]),
                    func=AT.Exp)
                nc.vector.tensor_tensor(
                    out=_ap_view(F8[:], nfeat, [[fw, ncg], [1, nheads]]),
                    in0=_ap_view(F8[:], nfeat, [[fw, ncg], [1, nheads]]),
                    in1=_ap_view(e2[:], 0, [[nheads, ncg], [1, nheads]]),
                    op=OP.max)
                hd = nfeat // nheads
                nc.vector.tensor_tensor(
                    out=_ap_view(F8[:], 0, [[fw, ncg], [hd, nheads], [1, hd]]),
                    in0=_ap_view(grow[:], 0,
                                 [[width, ncg], [hd, nheads], [1, hd]]),
                    in1=_ap_view(F8[:], nfeat,
                                 [[fw, ncg], [1, nheads], [0, hd]]),
                    op=OP.mult)
                for jj in range(ncg):
                    s, k, _tb2 = meta[c0 + jj]
                    nc.tensor.matmul(
                        out=ps_cur[0][:, :ps_width],
                        lhsT=S8[:, jj * P:(jj + 1) * P],
                        rhs=F8[:, jj * fw:jj * fw + ps_width],
                        start=(k == 0), stop=(k == Ktot[s] - 1))
                    if k == Ktot[s] - 1:
                        slot_epilogue(s, ps_cur[0])

        # L1 epilogue: h -> transpose -> G2 rows + AD2
        def epi1(s, ps):
            rc = epool.tile([P, H], f32, tag="rc", name="rc")
            nc.vector.tensor_scalar_add(out=rc[:], in0=ps[:, F1:F1 + H],
                                        scalar1=EPS)
            rc2 = epool.tile([P, H], f32, tag="rc2", name="rc2")
            nc.vector.reciprocal(out=rc2[:], in_=rc[:])
            hm = epool.tile([P, F1], f32, tag="hm", name="hm")
            nc.vector.tensor_tensor(
                out=_ap_view(hm[:], 0, [[HID, H], [1, HID]]),
                in0=_ap_view(ps[:, :F1], 0, [[HID, H], [1, HID]]),
                in1=_ap_view(rc2[:], 0, [[1, H], [0, HID]]),
                op=OP.mult)
            hb = epool.tile([P, F1], f32, tag="hb", name="hb")
            nc.vector.tensor_tensor(out=hb[:], in0=hm[:], in1=b1t[:], op=OP.add)
            hr = epool.tile([P, F1], f32, tag="hr", name="hr")
            nc.scalar.activation(out=hr[:], in_=hb[:], func=AT.Relu)
            pt1 = pst.tile([P, P], f32, tag="tr", name="pt1")
            nc.tensor.transpose(out=pt1[:], in_=hr[:, :P], identity=ident[:])
            pt2 = pst.tile([F1 - P, P], f32, tag="tr2", name="pt2")
            nc.tensor.transpose(out=pt2[:], in_=hr[:, P:F1], identity=ident[:])
            ht1 = epool.tile([P, P], f32, tag="ht1", name="ht1")
            nc.vector.tensor_copy(out=ht1[:], in_=pt1[:])
            ht2 = epool.tile([F1 - P, P], f32, tag="ht2", name="ht2")
            nc.vector.tensor_copy(out=ht2[:], in_=pt2[:])
            pg = psum.tile([P, 200], f32, tag="mm", name="pg")
            nc.tensor.matmul(out=pg[:, :66], lhsT=ht1[:], rhs=rhs2_lo[:],
                             start=True, stop=False)
            nc.tensor.matmul(out=pg[:, :66], lhsT=ht2[:], rhs=rhs2_hi[:],
                             start=False, stop=True)
            g2 = epool.tile([P, G2W], bf16, tag="g2", name="g2")
            nc.vector.tensor_copy(out=g2[:, :F2], in_=pg[:, :F2])
            g2f = g2[:].bitcast(f32)
            nc.vector.tensor_copy(out=g2f[:, 32:33], in_=pg[:, F2:F2 + 1])
            ad2w = epool.tile([P, 2], f32, tag="ad2w", name="ad2w")
            nc.vector.tensor_copy(out=ad2w[:, :1], in_=pg[:, F2 + 1:F2 + 2])
            nc.sync.dma_start(out=G2L[s * P:(s + 1) * P, :], in_=g2[:])
            nc.sync.dma_start(out=AD2L[s * P:(s + 1) * P, :1], in_=ad2w[:, :1])

        edge_layer(L1, G1a, G1b, G1W, F1, 96, AD1, 3, 0,
                   DREL1, DRELT1, IDXW1, BLKI, F1 + H, epi1)

        # ---------------- AllGather ----------------
        nc.gpsimd.collective_compute(
            "AllGather", mybir.AluOpType.bypass,
            replica_groups=[list(range(NCORES))],
            ins=[G2L.ap().opt()], outs=[G2F.ap().opt()])
        nc.gpsimd.collective_compute(
            "AllGather", mybir.AluOpType.bypass,
            replica_groups=[list(range(NCORES))],
            ins=[AD2L.ap().opt()], outs=[AD2F.ap().opt()])

        # ---------------- layer 2 ----------------
        def epi2(s, ps):
            rc = epool.tile([P, 1], f32, tag="rcB", name="rcB")
            nc.vector.tensor_scalar_add(out=rc[:], in0=ps[:, F2:F2 + 1],
                                        scalar1=EPS)
            rc2 = epool.tile([P, 1], f32, tag="rcB2", name="rcB2")
            nc.vector.reciprocal(out=rc2[:], in_=rc[:])
            om = epool.tile([P, F2], f32, tag="om", name="om")
            nc.vector.tensor_tensor(out=om[:], in0=ps[:, :F2],
                                    in1=rc2[:].to_broadcast([P, F2]),
                                    op=OP.mult)
            ob = epool.tile([P, F2], f32, tag="ob", name="ob")
            nc.vector.tensor_tensor(out=ob[:], in0=om[:], in1=b2t[:], op=OP.add)
            orl = epool.tile([P, F2], f32, tag="orl", name="orl")
            nc.scalar.activation(out=orl[:], in_=ob[:], func=AT.Relu)
            nc.sync.dma_start(out=OUT[s * P:(s + 1) * P, :], in_=orl[:])

        # G2F views for the two index halves (offsets stay < 2^24 bytes)
        g2fa = G2F[0:HALF, :]
        g2fb = G2F[HALF:NROWC * NCORES, :]
        edge_layer(L2, g2fa, g2fb, G2W, F2, 32, AD2F, 1, 0,
                   DREL2, DRELT2, IDXW2, BLKI2, F2 + 1, epi2)

    nc.compile()
    return nc


def _get_compiled(key, layers):
    if key not in _compiled:
        _compiled[key] = _build(layers[0], layers[1])
    return _compiled[key]


def run(inputs, **runkw):
    from concourse import bass_utils

    key, layers, shared, percore = _host_prep(inputs)
    nc = _get_compiled(key, layers)
    in_maps = []
    for c in range(NCORES):
        m = dict(shared)
        m.update(percore[c])
        in_maps.append(m)
    res = bass_utils.run_bass_kernel_spmd(
        nc, in_maps, core_ids=list(range(NCORES)), **runkw)
    return res


def assemble(results):
    out = np.empty((N, F2), dtype=np.float32)
    for c in range(NCORES):
        out[c * NPC:(c + 1) * NPC] = results[c]["out"][:NPC]
    return out


def kernel(**inputs):
    res = run(inputs)
    return assemble(res.results)


# revision 48
# speedup vs baseline: 1.9336x; 1.0947x over previous
"""GAT 2-layer kernel for Trainium2, 8 NeuronCores (SPMD, dst-sharded).

Strategy (v3):
  - Destination-node sharding: core c owns nodes [c*6250,(c+1)*6250); edges bucketed
    into per-128-dst-node "slots", padded to 128-edge chunks.
  - Stage A (replicated): per 128-node tile one matmul computes
    [x@W1 (192) | alpha_src (3) | alpha_dst (3)]; xw+as go to a bf16 gather table
    G1 (512B rows: 192 bf16 xw + 3 f32 alpha_src bit-packed + pad), ad to slim f32
    table AD1. G1 is split into two <=32768-row tensors (dma_gather int16 index
    limit, 16MiB ucode offset limit).
  - Edge phase per layer: per <=8-chunk group one dma_gather (1024 row gathers/op)
    pulls source rows; one-hot S (DVE is_equal vs iota) segment-reduces
    exp(logit)-weighted features AND the softmax denominators in one per-slot PSUM
    accumulation (normalization pulled out of the sum; logits are O(5), no
    max-subtraction needed). alpha_dst[dst] is expanded edge-wise on the PE:
    one-hot-transpose S_T (built by K=1 ones-matmul broadcast of dst_rel + DVE
    compare) times the slot's alpha_dst block (gathered once per slot).
  - Per-slot epilogue: h = relu(sum/(denom+eps) + bias1); PE-transpose h and
    immediately emit G2 rows [h@W2 (64) bf16 | as2 f32] and slim AD2; AllGather
    both; layer 2 repeats the edge phase (1 head) against G2F views.
"""
import sys

sys.path.insert(0, "/opt/trn_rl_repo")
import numpy as np
import ml_dtypes

N = 50000
D = 128
HID = 64
H = 3
F1 = 192
F2 = 64
NCORES = 8
NPC = N // NCORES          # 6250 nodes per core
P = 128
NBLK = (NPC + P - 1) // P  # 49 slots per core
NT = (N + P - 1) // P      # 391 stage-A node tiles
NROW1 = NT * P             # 50048 G1 rows
HALF = 32768               # dma_gather int16 index limit
G1W = 256                  # bf16 cols: xw(192) | as f32 x3 (bf16 192:198) | pad
G2W = 128                  # bf16 cols: xw2(64) | as2 f32 (bf16 64:66) | pad
NROWC = NBLK * P           # 6272 rows per core shard
SLOPE = 0.2
EPS = 1e-16
GRP = 8                    # max chunks per dma_gather / op group
SUB = 4                    # chunks per S_T broadcast matmul (512 psum cols)

_compiled = {}


def _chunkize(src_key, dst, order_all):
    """Bucket edges per (core, slot), split by src_key half, pad to 128.
    Returns per-core flat arrays + compile-time chunk structure (shared)."""
    core = dst // NPC
    rel = dst % NPC
    slot = rel // P
    half = (src_key >= HALF).astype(np.int64)
    # counts[core, slot, half]
    counts = np.zeros((NCORES, NBLK, 2), dtype=np.int64)
    np.add.at(counts, (core, slot, half), 1)
    Ka = np.ceil(counts[:, :, 0] / P).astype(np.int64).max(axis=0)
    Kb = np.ceil(counts[:, :, 1] / P).astype(np.int64).max(axis=0)
    return Ka, Kb


def _host_prep(inputs):
    x = np.asarray(inputs["x"], dtype=np.float32)
    ei = np.asarray(inputs["edge_index"])
    W1 = np.asarray(inputs["W1"], dtype=np.float32)
    as1 = np.asarray(inputs["att_src1"], dtype=np.float32)
    ad1 = np.asarray(inputs["att_dst1"], dtype=np.float32)
    b1 = np.asarray(inputs["bias1"], dtype=np.float32)
    W2 = np.asarray(inputs["W2"], dtype=np.float32)
    as2 = np.asarray(inputs["att_src2"], dtype=np.float32)
    ad2 = np.asarray(inputs["att_dst2"], dtype=np.float32)
    b2 = np.asarray(inputs["bias2"], dtype=np.float32)

    loops = np.arange(N, dtype=np.int64)
    src = np.concatenate([ei[0].astype(np.int64), loops])
    dst = np.concatenate([ei[1].astype(np.int64), loops])
    order = np.argsort(dst, kind="stable")
    src = src[order]
    dst = dst[order]
    g2row = (src // NPC) * NROWC + (src % NPC)

    # chunk structure per layer (uniform across cores)
    Ka1, Kb1 = _chunkize(src, dst, None)
    Ka2, Kb2 = _chunkize(g2row, dst, None)

    def build_layer(key):
        Ka, Kb = (Ka1, Kb1) if key == 1 else (Ka2, Kb2)
        skey = src if key == 1 else g2row
        NCH = int((Ka + Kb).sum())
        # chunk meta: (slot, k_in_slot, table) in processing order
        # b-table chunks first: their gathers only depend on the (smaller,
        # first-written) G1b table, overlapping the tail of stage A
        meta = []
        for s in range(NBLK):
            k = 0
            for _ in range(int(Kb[s])):
                meta.append((s, k, 1)); k += 1
            for _ in range(int(Ka[s])):
                meta.append((s, k, 0)); k += 1
        # gather ops: runs of <=GRP same-table consecutive chunks
        ops = []   # (chunk_start, n_chunks, table)
        i = 0
        while i < NCH:
            t = meta[i][2]
            j = i
            while j < NCH and j - i < GRP and meta[j][2] == t:
                j += 1
            ops.append((i, j - i, t))
            i = j
        NOPS = len(ops)

        EPAD = NCH * P
        SRCK = np.zeros((NCORES, EPAD), dtype=np.int64)
        DREL = np.full((NCORES, EPAD), 255.0, dtype=np.float32)
        for c in range(NCORES):
            base_node = c * NPC
            # per-slot edge ranges (dst-sorted => contiguous)
            cb = 0
            for s in range(NBLK):
                blo = base_node + s * P
                bhi = min(blo + P, base_node + NPC)
                lo = np.searchsorted(dst, blo, side="left")
                hi = np.searchsorted(dst, bhi, side="left")
                sk = skey[lo:hi]
                dr = (dst[lo:hi] - blo).astype(np.float32)
                a_mask = sk < HALF
                for which, KK, pad in ((~a_mask, Kb[s], HALF),
                                       (a_mask, Ka[s], 0)):
                    cnt = int(which.sum())
                    pos = cb * P
                    SRCK[c, pos:pos + cnt] = sk[which]
                    # pad indices must stay valid for the table half
                    SRCK[c, pos + cnt:(cb + int(KK)) * P] = pad
                    DREL[c, pos:pos + cnt] = dr[which]
                    cb += int(KK)
        # device arrays
        DREL_t = np.ascontiguousarray(
            DREL.reshape(NCORES, NCH, P).transpose(0, 2, 1))
        DRELT = np.ascontiguousarray(DREL.reshape(NCORES, 1, EPAD))
        # wrapped int16 indices per gather op, [128, NOPS*64]
        IDXW = np.zeros((NCORES, P, NOPS * GRP * 8), dtype=np.int16)
        for c in range(NCORES):
            for o, (c0, ncg, t) in enumerate(ops):
                iv = SRCK[c, c0 * P:(c0 + ncg) * P] - (HALF if t else 0)
                w = iv.reshape(-1, 16).T.astype(np.int16)  # [16, n/16]
                IDXW[c, :, o * GRP * 8: o * GRP * 8 + w.shape[1]] = \
                    np.tile(w, (8, 1))
        return dict(NCH=NCH, meta=meta, ops=ops, NOPS=NOPS,
                    Ktot=[int(Ka[s] + Kb[s]) for s in range(NBLK)],
                    DREL=DREL_t, DRELT=DRELT, IDXW=IDXW)

    L1 = build_layer(1)
    L2 = build_layer(2)

    # per-slot block-node gather indices (alpha_dst blocks)
    BLKI = np.zeros((NCORES, P, NBLK), dtype=np.int32)
    BLKI2 = np.zeros((NCORES, P, NBLK), dtype=np.int32)
    for c in range(NCORES):
        for s in range(NBLK):
            nodes = np.minimum(c * NPC + s * P + np.arange(P), N - 1)
            BLKI[c, :, s] = nodes
            BLKI2[c, :, s] = (nodes // NPC) * NROWC + (nodes % NPC)

    xT = np.zeros((D, NROW1), dtype=np.float32)
    xT[:, :N] = x.T
    A1 = np.zeros((F1, 6), dtype=np.float32)
    for h in range(H):
        A1[h * HID:(h + 1) * HID, h] = as1[h]
        A1[h * HID:(h + 1) * HID, 3 + h] = ad1[h]
    A2 = np.stack([as2[0], ad2[0]], axis=1).astype(np.float32)

    shared = {
        "xT": xT,
        "W1": np.ascontiguousarray(W1),
        "W1T": np.ascontiguousarray(W1.T),
        "A1": A1,
        "W2": np.ascontiguousarray(W2),
        "W2T": np.ascontiguousarray(W2.T),
        "A2": A2,
        "B1": np.ascontiguousarray(np.broadcast_to(b1, (P, F1))),
        "B2": np.ascontiguousarray(np.broadcast_to(b2, (P, F2))),
        "IOTA": np.ascontiguousarray(
            np.broadcast_to(np.arange(P, dtype=np.float32), (P, P))),
        "IOTAC": np.arange(P, dtype=np.float32).reshape(P, 1),
    }
    percore = []
    for c in range(NCORES):
        percore.append({
            "DREL1": L1["DREL"][c], "DRELT1": L1["DRELT"][c],
            "IDXW1": L1["IDXW"][c],
            "DREL2": L2["DREL"][c], "DRELT2": L2["DRELT"][c],
            "IDXW2": L2["IDXW"][c],
            "BLKI": BLKI[c], "BLKI2": BLKI2[c],
        })
    key = (tuple(L1["Ktot"]), tuple(x[0] for x in L1["ops"]),
           tuple(x[1] for x in L1["ops"]), tuple(x[2] for x in L1["ops"]),
           tuple(L2["Ktot"]), tuple(x[0] for x in L2["ops"]),
           tuple(x[1] for x in L2["ops"]), tuple(x[2] for x in L2["ops"]))
    return key, (L1, L2), shared, percore


def _ap_view(ap, extra_offset, free_dims):
    import concourse.bass as bass

    return bass.AP(
        tensor=ap.tensor, offset=ap.offset + extra_offset,
        ap=[list(ap.ap[0])] + [list(d) for d in free_dims],
    )


def _build(L1, L2):
    import concourse.bass as bass
    import concourse.bacc as bacc
    import concourse.tile as tile
    from concourse import mybir
    from concourse.masks import make_identity
    from concourse.library_config import mlp
    from contextlib import ExitStack

    f32 = mybir.dt.float32
    bf16 = mybir.dt.bfloat16
    i32 = mybir.dt.int32
    i16 = mybir.dt.int16
    AT = mybir.ActivationFunctionType
    OP = mybir.AluOpType
    IOA = bass.IndirectOffsetOnAxis

    nc = bacc.Bacc("TRN2", target_bir_lowering=False, debug=False,
                   num_devices=NCORES, num_swdge_queues=4)

    xT = nc.dram_tensor("xT", [D, NROW1], f32, kind="ExternalInput")
    W1 = nc.dram_tensor("W1", [D, F1], f32, kind="ExternalInput")
    W1T = nc.dram_tensor("W1T", [F1, D], f32, kind="ExternalInput")
    A1 = nc.dram_tensor("A1", [F1, 6], f32, kind="ExternalInput")
    W2 = nc.dram_tensor("W2", [F1, F2], f32, kind="ExternalInput")
    W2T = nc.dram_tensor("W2T", [F2, F1], f32, kind="ExternalInput")
    A2 = nc.dram_tensor("A2", [F2, 2], f32, kind="ExternalInput")
    B1 = nc.dram_tensor("B1", [P, F1], f32, kind="ExternalInput")
    B2 = nc.dram_tensor("B2", [P, F2], f32, kind="ExternalInput")
    IOTA = nc.dram_tensor("IOTA", [P, P], f32, kind="ExternalInput")
    IOTAC = nc.dram_tensor("IOTAC", [P, 1], f32, kind="ExternalInput")
    DREL1 = nc.dram_tensor("DREL1", [P, L1["NCH"]], f32, kind="ExternalInput")
    DRELT1 = nc.dram_tensor("DRELT1", [1, L1["NCH"] * P], f32,
                            kind="ExternalInput")
    IDXW1 = nc.dram_tensor("IDXW1", [P, L1["NOPS"] * GRP * 8], i16,
                           kind="ExternalInput")
    DREL2 = nc.dram_tensor("DREL2", [P, L2["NCH"]], f32, kind="ExternalInput")
    DRELT2 = nc.dram_tensor("DRELT2", [1, L2["NCH"] * P], f32,
                            kind="ExternalInput")
    IDXW2 = nc.dram_tensor("IDXW2", [P, L2["NOPS"] * GRP * 8], i16,
                           kind="ExternalInput")
    BLKI = nc.dram_tensor("BLKI", [P, NBLK], i32, kind="ExternalInput")
    BLKI2 = nc.dram_tensor("BLKI2", [P, NBLK], i32, kind="ExternalInput")
    OUT = nc.dram_tensor("out", [NROWC, F2], f32, kind="ExternalOutput")

    G1a = nc.dram_tensor("G1a", [HALF, G1W], bf16, kind="Internal")
    G1b = nc.dram_tensor("G1b", [NROW1 - HALF, G1W], bf16, kind="Internal")
    AD1 = nc.dram_tensor("AD1", [NROW1, 4], f32, kind="Internal")
    G2L = nc.dram_tensor("G2L", [NROWC, G2W], bf16, kind="Internal")
    AD2L = nc.dram_tensor("AD2L", [NROWC, 2], f32, kind="Internal")
    G2F = nc.dram_tensor("G2F", [NROWC * NCORES, G2W], bf16,
                         addr_space="Shared", kind="Internal")
    AD2F = nc.dram_tensor("AD2F", [NROWC * NCORES, 2], f32,
                          addr_space="Shared", kind="Internal")

    with tile.TileContext(nc) as tc, ExitStack() as ctx:
        consts = ctx.enter_context(tc.tile_pool(name="consts", bufs=1))
        sbA = ctx.enter_context(tc.tile_pool(name="sbA", bufs=8))
        psum = ctx.enter_context(tc.tile_pool(name="psum", bufs=3, space="PSUM"))
        psbc = ctx.enter_context(tc.tile_pool(name="psbc", bufs=1, space="PSUM"))
        psad = ctx.enter_context(tc.tile_pool(name="psad", bufs=2, space="PSUM"))
        pst = ctx.enter_context(tc.tile_pool(name="pst", bufs=1, space="PSUM"))
        gpool = ctx.enter_context(tc.tile_pool(name="gpool", bufs=6))
        fpool = ctx.enter_context(tc.tile_pool(name="fpool", bufs=6))
        spool = ctx.enter_context(tc.tile_pool(name="spool", bufs=6))
        ipool = ctx.enter_context(tc.tile_pool(name="ipool", bufs=6))
        epool = ctx.enter_context(tc.tile_pool(name="epool", bufs=4))

        nc.gpsimd.load_library(mlp)

        # ---------------- constants / weight prep ----------------
        iota = consts.tile([P, P], f32)
        nc.sync.dma_start(out=iota[:], in_=IOTA[:])
        iotac = consts.tile([P, 1], f32)
        nc.sync.dma_start(out=iotac[:], in_=IOTAC[:])
        ones1 = consts.tile([1, P], f32)
        nc.vector.memset(ones1[:], 1.0)
        b1t = consts.tile([P, F1], f32)
        nc.sync.dma_start(out=b1t[:], in_=B1[:])
        b2t = consts.tile([P, F2], f32)
        nc.sync.dma_start(out=b2t[:], in_=B2[:])
        ident = consts.tile([P, P], f32)
        make_identity(nc, ident[:])

        rhs1 = consts.tile([P, 198], f32)
        nc.sync.dma_start(out=rhs1[:, :F1], in_=W1[:])
        w1t_a = consts.tile([P, D], f32)
        nc.sync.dma_start(out=w1t_a[:], in_=W1T[0:P, :])
        w1t_b = consts.tile([F1 - P, D], f32)
        nc.sync.dma_start(out=w1t_b[:], in_=W1T[P:F1, :])
        a1_a = consts.tile([P, 6], f32)
        nc.sync.dma_start(out=a1_a[:], in_=A1[0:P, :])
        a1_b = consts.tile([F1 - P, 6], f32)
        nc.sync.dma_start(out=a1_b[:], in_=A1[P:F1, :])
        pu = pst.tile([P, P], f32, tag="tr")
        nc.tensor.matmul(out=pu[:, :6], lhsT=w1t_a[:], rhs=a1_a[:],
                         start=True, stop=False)
        nc.tensor.matmul(out=pu[:, :6], lhsT=w1t_b[:], rhs=a1_b[:],
                         start=False, stop=True)
        nc.vector.tensor_copy(out=rhs1[:, F1:F1 + 6], in_=pu[:, :6])

        w2t = consts.tile([F2, F1], f32)
        nc.sync.dma_start(out=w2t[:], in_=W2T[:])
        a2t = consts.tile([F2, 2], f32)
        nc.sync.dma_start(out=a2t[:], in_=A2[:])
        rhs2_lo = consts.tile([P, 66], f32)
        nc.sync.dma_start(out=rhs2_lo[:, :F2], in_=W2[0:P, :])
        rhs2_hi = consts.tile([F1 - P, 66], f32)
        nc.sync.dma_start(out=rhs2_hi[:, :F2], in_=W2[P:F1, :])
        pu2a = pst.tile([P, P], f32, tag="tr")
        nc.tensor.matmul(out=pu2a[:, :2], lhsT=w2t[:, 0:P], rhs=a2t[:],
                         start=True, stop=True)
        nc.vector.tensor_copy(out=rhs2_lo[:, F2:F2 + 2], in_=pu2a[:, :2])
        pu2b = pst.tile([F1 - P, P], f32, tag="tr2")
        nc.tensor.matmul(out=pu2b[:, :2], lhsT=w2t[:, P:F1], rhs=a2t[:],
                         start=True, stop=True)
        nc.vector.tensor_copy(out=rhs2_hi[:, F2:F2 + 2], in_=pu2b[:, :2])

        # ---------------- stage A (G1b tiles first) ----------------
        for t in list(range(HALF // P, NT)) + list(range(HALF // P)):
            xt = sbA.tile([P, P], f32, tag="xt")
            nc.sync.dma_start(out=xt[:], in_=xT[:, t * P:(t + 1) * P])
            pa = psum.tile([P, 200], f32, tag="mm")
            nc.tensor.matmul(out=pa[:, :198], lhsT=xt[:], rhs=rhs1[:],
                             start=True, stop=True)
            gbf = sbA.tile([P, G1W], bf16, tag="gbf")
            nc.scalar.activation(out=gbf[:, :F1], in_=pa[:, :F1], func=AT.Copy)
            gf32 = gbf[:].bitcast(f32)
            nc.vector.tensor_copy(out=gf32[:, 96:99], in_=pa[:, F1:F1 + 3])
            adw = sbA.tile([P, 4], f32, tag="adw")
            nc.vector.tensor_copy(out=adw[:, :3], in_=pa[:, F1 + 3:F1 + 6])
            if t < HALF // P:
                nc.scalar.dma_start(out=G1a[t * P:(t + 1) * P, :], in_=gbf[:])
            else:
                tb = t - HALF // P
                nc.scalar.dma_start(out=G1b[tb * P:(tb + 1) * P, :], in_=gbf[:])
            nc.scalar.dma_start(out=AD1[t * P:(t + 1) * P, :3], in_=adw[:, :3])

        # ---------------- generic edge phase ----------------
        def edge_layer(LM, TBLa, TBLb, width, nfeat, as_f32col, ADT, adw_,
                       adcol, dreli, drelti, idxwi, blki, ps_width,
                       slot_epilogue):
            nheads = ps_width - nfeat
            NCH = LM["NCH"]
            meta = LM["meta"]
            ops = LM["ops"]
            Ktot = LM["Ktot"]
            blkit = consts.tile([P, NBLK], i32, name=f"blkit{nfeat}")
            nc.sync.dma_start(out=blkit[:], in_=blki[:])
            ps_cur = [None]
            adb_cur = [None]
            fw = nfeat + nheads  # F8 row width

            def new_slot(s):
                adb = epool.tile([P, 4], f32, tag="adb", name="adb")
                nc.gpsimd.indirect_dma_start(
                    out=adb[:, :adw_], out_offset=None, in_=ADT[:],
                    in_offset=IOA(ap=blkit[:, s:s + 1], axis=0))
                adbh = epool.tile([P, 4], bf16, tag="adbh", name="adbh")
                nc.vector.tensor_copy(out=adbh[:, :adw_], in_=adb[:, :adw_])
                adb_cur[0] = adbh
                ps_cur[0] = psum.tile([P, 200], f32, tag="mm", name="ps_slot")

            for o, (c0, ncg, tb) in enumerate(ops):
                idxt = ipool.tile([P, GRP * 8], i16, tag="idxt", name="idxt")
                nc.sync.dma_start(
                    out=idxt[:, :ncg * 8],
                    in_=idxwi[:, o * GRP * 8:o * GRP * 8 + ncg * 8])
                drt = ipool.tile([P, GRP], f32, tag="drt", name="drt")
                nc.sync.dma_start(out=drt[:, :ncg],
                                  in_=dreli[:, c0:c0 + ncg])
                grow = gpool.tile([P, GRP, width], bf16, tag="grow",
                                  name="grow")
                nidx = ncg * P
                nc.gpsimd.dma_gather(
                    grow[:, :ncg, :], (TBLb if tb else TBLa)[:],
                    idxt[:, :ncg * 8], nidx, nidx, width,
                    queue_num=o % 4)
                # S: [e_part, chunk, d] one-hot
                S8 = spool.tile([P, GRP * P], bf16, tag="s8", name="s8")
                nc.vector.tensor_tensor(
                    out=_ap_view(S8[:], 0, [[P, ncg], [1, P]]),
                    in0=_ap_view(drt[:], 0, [[1, ncg], [0, P]]),
                    in1=_ap_view(iota[:], 0, [[0, ncg], [1, P]]),
                    op=OP.is_equal)
                # S_T + alpha_dst expansion (per SUB-chunk batches)
                adp = psad.tile([P, GRP * nheads], f32, tag="adp", name="adp")
                j = 0
                while j < ncg:
                    sb = min(SUB, ncg - j)
                    drl = ipool.tile([1, SUB * P], f32, tag="drl", name="drl")
                    nc.scalar.dma_start(
                        out=drl[:, :sb * P],
                        in_=drelti[:, (c0 + j) * P:(c0 + j + sb) * P])
                    pbc = psbc.tile([P, SUB * P], f32, tag="bc", name="pbc")
                    nc.tensor.matmul(out=pbc[:, :sb * P], lhsT=ones1[:],
                                     rhs=drl[:, :sb * P], start=True, stop=True)
                    st8 = spool.tile([P, SUB * P], bf16, tag="st8", name="st8")
                    nc.vector.tensor_scalar(
                        out=st8[:, :sb * P], in0=pbc[:, :sb * P],
                        scalar1=iotac[:, :1], scalar2=None, op0=OP.is_equal)
                    for jj in range(sb):
                        s, k, _tb2 = meta[c0 + j + jj]
                        if k == 0:
                            new_slot(s)
                        nc.tensor.matmul(
                            out=adp[:, (j + jj) * nheads:(j + jj + 1) * nheads],
                            lhsT=st8[:, jj * P:(jj + 1) * P],
                            rhs=adb_cur[0][:, adcol:adcol + nheads],
                            start=True, stop=True)
                    j += sb
                # logits -> exp -> weighted features
                growf = grow[:].bitcast(f32)
                t8 = epool.tile([P, GRP * nheads], f32, tag="t8", name="t8")
                nc.vector.tensor_tensor(
                    out=_ap_view(t8[:], 0, [[nheads, ncg], [1, nheads]]),
                    in0=_ap_view(growf, as_f32col,
                                 [[width // 2, ncg], [1, nheads]]),
                    in1=_ap_view(adp[:], 0, [[nheads, ncg], [1, nheads]]),
                    op=OP.add)
                # exp(lrelu(t)) == max(exp(t), exp(SLOPE*t)) exactly
                e2 = epool.tile([P, GRP * nheads], f32, tag="r8", name="e2")
                nc.scalar.activation(out=e2[:, :ncg * nheads],
                                     in_=t8[:, :ncg * nheads],
                                     func=AT.Exp, scale=SLOPE)
                F8 = fpool.tile([P, GRP * fw], bf16, tag="f8", name="f8")
                nc.scalar.activation(
                    out=_ap_view(F8[:], nfeat, [[fw, ncg], [1, nheads]]),
                    in_=_ap_view(t8[:], 0, [[nheads, ncg], [1, nheads]]),
                    func=AT.Exp)
                nc.vector.tensor_tensor(
                    out=_ap_view(F8[:], nfeat, [[fw, ncg], [1, nheads]]),
                    in0=_ap_view(F8[:], nfeat, [[fw, ncg], [1, nheads]]),
                    in1=_ap_view(e2[:], 0, [[nheads, ncg], [1, nheads]]),
                    op=OP.max)
                hd = nfeat // nheads
                nc.vector.tensor_tensor(
                    out=_ap_view(F8[:], 0, [[fw, ncg], [hd, nheads], [1, hd]]),
                    in0=_ap_view(grow[:], 0,
                                 [[width, ncg], [hd, nheads], [1, hd]]),
                    in1=_ap_view(F8[:], nfeat,
                                 [[fw, ncg], [1, nheads], [0, hd]]),
                    op=OP.mult)
                for jj in range(ncg):
                    s, k, _tb2 = meta[c0 + jj]
                    nc.tensor.matmul(
                        out=ps_cur[0][:, :ps_width],
                        lhsT=S8[:, jj * P:(jj + 1) * P],
                        rhs=F8[:, jj * fw:jj * fw + ps_width],
                        start=(k == 0), stop=(k == Ktot[s] - 1))
                    if k == Ktot[s] - 1:
                        slot_epilogue(s, ps_cur[0])

        # L1 epilogue: h -> transpose -> G2 rows + AD2
        def epi1(s, ps):
            rc = epool.tile([P, H], f32, tag="rc", name="rc")
            nc.vector.tensor_scalar_add(out=rc[:], in0=ps[:, F1:F1 + H],
                                        scalar1=EPS)
            rc2 = epool.tile([P, H], f32, tag="rc2", name="rc2")
            nc.vector.reciprocal(out=rc2[:], in_=rc[:])
            hm = epool.tile([P, F1], f32, tag="hm", name="hm")
            nc.vector.tensor_tensor(
                out=_ap_view(hm[:], 0, [[HID, H], [1, HID]]),
                in0=_ap_view(ps[:, :F1], 0, [[HID, H], [1, HID]]),
                in1=_ap_view(rc2[:], 0, [[1, H], [0, HID]]),
                op=OP.mult)
            hb = epool.tile([P, F1], f32, tag="hb", name="hb")
            nc.vector.tensor_tensor(out=hb[:], in0=hm[:], in1=b1t[:], op=OP.add)
            hr = epool.tile([P, F1], f32, tag="hr", name="hr")
            nc.scalar.activation(out=hr[:], in_=hb[:], func=AT.Relu)
            pt1 = pst.tile([P, P], f32, tag="tr", name="pt1")
            nc.tensor.transpose(out=pt1[:], in_=hr[:, :P], identity=ident[:])
            pt2 = pst.tile([F1 - P, P], f32, tag="tr2", name="pt2")
            nc.tensor.transpose(out=pt2[:], in_=hr[:, P:F1], identity=ident[:])
            ht1 = epool.tile([P, P], f32, tag="ht1", name="ht1")
            nc.vector.tensor_copy(out=ht1[:], in_=pt1[:])
            ht2 = epool.tile([F1 - P, P], f32, tag="ht2", name="ht2")
            nc.vector.tensor_copy(out=ht2[:], in_=pt2[:])
            pg = psum.tile([P, 200], f32, tag="mm", name="pg")
            nc.tensor.matmul(out=pg[:, :66], lhsT=ht1[:], rhs=rhs2_lo[:],
                             start=True, stop=False)
            nc.tensor.matmul(out=pg[:, :66], lhsT=ht2[:], rhs=rhs2_hi[:],
                             start=False, stop=True)
            g2 = epool.tile([P, G2W], bf16, tag="g2", name="g2")
            nc.vector.tensor_copy(out=g2[:, :F2], in_=pg[:, :F2])
            g2f = g2[:].bitcast(f32)
            nc.vector.tensor_copy(out=g2f[:, 32:33], in_=pg[:, F2:F2 + 1])
            ad2w = epool.tile([P, 2], f32, tag="ad2w", name="ad2w")
            nc.vector.tensor_copy(out=ad2w[:, :1], in_=pg[:, F2 + 1:F2 + 2])
            nc.sync.dma_start(out=G2L[s * P:(s + 1) * P, :], in_=g2[:])
            nc.sync.dma_start(out=AD2L[s * P:(s + 1) * P, :1], in_=ad2w[:, :1])

        edge_layer(L1, G1a, G1b, G1W, F1, 96, AD1, 3, 0,
                   DREL1, DRELT1, IDXW1, BLKI, F1 + H, epi1)

        # ---------------- AllGather ----------------
        nc.gpsimd.collective_compute(
            "AllGather", mybir.AluOpType.bypass,
            replica_groups=[list(range(NCORES))],
            ins=[G2L.ap().opt()], outs=[G2F.ap().opt()])
        nc.gpsimd.collective_compute(
            "AllGather", mybir.AluOpType.bypass,
            replica_groups=[list(range(NCORES))],
            ins=[AD2L.ap().opt()], outs=[AD2F.ap().opt()])

        # ---------------- layer 2 ----------------
        def epi2(s, ps):
            rc = epool.tile([P, 1], f32, tag="rcB", name="rcB")
            nc.vector.tensor_scalar_add(out=rc[:], in0=ps[:, F2:F2 + 1],
                                        scalar1=EPS)
            rc2 = epool.tile([P, 1], f32, tag="rcB2", name="rcB2")
            nc.vector.reciprocal(out=rc2[:], in_=rc[:])
            om = epool.tile([P, F2], f32, tag="om", name="om")
            nc.vector.tensor_tensor(out=om[:], in0=ps[:, :F2],
                                    in1=rc2[:].to_broadcast([P, F2]),
                                    op=OP.mult)
            ob = epool.tile([P, F2], f32, tag="ob", name="ob")
            nc.vector.tensor_tensor(out=ob[:], in0=om[:], in1=b2t[:], op=OP.add)
            orl = epool.tile([P, F2], f32, tag="orl", name="orl")
            nc.scalar.activation(out=orl[:], in_=ob[:], func=AT.Relu)
            nc.sync.dma_start(out=OUT[s * P:(s + 1) * P, :], in_=orl[:])

        # G2F views for the two index halves (offsets stay < 2^24 bytes)
        g2fa = G2F[0:HALF, :]
        g2fb = G2F[HALF:NROWC * NCORES, :]
        edge_layer(L2, g2fa, g2fb, G2W, F2, 32, AD2F, 1, 0,
                   DREL2, DRELT2, IDXW2, BLKI2, F2 + 1, epi2)

    nc.compile()
    return nc


def _get_compiled(key, layers):
    if key not in _compiled:
        _compiled[key] = _build(layers[0], layers[1])
    return _compiled[key]


def run(inputs, **runkw):
    from concourse import bass_utils

    key, layers, shared, percore = _host_prep(inputs)
    nc = _get_compiled(key, layers)
    in_maps = []
    for c in range(NCORES):
        m = dict(shared)
        m.update(percore[c])
        in_maps.append(m)
    res = bass_utils.run_bass_kernel_spmd(
        nc, in_maps, core_ids=list(range(NCORES)), **runkw)
    return res


def assemble(results):
    out = np.empty((N, F2), dtype=np.float32)
    for c in range(NCORES):
        out[c * NPC:(c + 1) * NPC] = results[c]["out"][:NPC]
    return out


def kernel(**inputs):
    res = run(inputs)
    return assemble(res.results)
